# revision 57
# baseline (speedup 1.0000x reference)
"""Multi-head self-attention (B=1, S=4096, DIM=768, H=12) on 8 Trainium2
NeuronCores.

Sharding: tensor-parallel over heads. Core c computes
  - full attention for head hA = c            (heads 0..7, all 4096 queries)
  - half attention for head hB = 8 + c//2     (heads 8..11, query half c%2)
Each core computes its own K/V projections for its two heads from x^T
streamed through SBUF once, runs attention fully on-chip, applies its heads'
slice of the output projection, and returns transposed partial projections
(bf16). The host sums per-core partials (the tensor-parallel all-reduce),
adds b_proj, and transposes back.

Fast path vs the fp32 baseline:
  - All matmuls in bf16 (x, QKV weights, Q/K/V, pt, proj weights).
  - exp(score) is split across two engines BY JOB: the scalar engine
    computes true exp for the sweep's first job; the vector engine handles
    the second job with a Schraudolph bit trick: Q is pre-scaled by
    C1 = 128/ln2 so the QK matmul emits scores in bf16-bits domain, then one
    tensor_scalar_add(+16261.5) with int16 output writes exp(s)*1.03 bit
    patterns directly into the bf16 pt tile (rel err ~3%, cancels in
    softmax's numerator/denominator except per-weight ripple). Each job's
    scores live in their own 1-bank PSUM tile (4-deep rotation) and PV
    consumption is deferred two k-tiles so neither engine's latency sits on
    the PE critical path.
  - PV is "flipped": stationary = pt q-chunk [128,128], moving = [V|1]
    [128,65], so each matmul streams only 65 output columns (the cost model
    charges output free size). Output lands q-on-partitions with the softmax
    denominator in column 64, so normalization is a per-partition reciprocal
    + broadcast multiply (no partition broadcast), then a cheap PE transpose
    restores [hd, q] for the output projection.
"""

import numpy as np
import ml_dtypes

DIM = 768
HEADS = 12
HD = 64
SCALE = HD ** (-0.5)
S = 4096
SH = 2048
NCORES = 8
KT = DIM // 128   # 6 k-tiles over the 768 contraction dim
NKT = S // 128    # 32 k-tiles over the 4096 sequence dim

C1 = 128.0 / np.log(2.0)   # schraudolph scale, folded into Q weights
C2V = 16256.0 + 5.5        # bf16 bits of 1.0 + centering, added at convert

_CACHE: dict = {}
SIM_STATIC = False  # profile scripts set True: TimelineSim can't eval branches


def _build_nc():
    import concourse.bacc as bacc
    import concourse.tile as tile
    from concourse import mybir
    from concourse.masks import make_identity

    f32 = mybir.dt.float32
    bf16 = mybir.dt.bfloat16
    i16 = mybir.dt.int16
    EXP = mybir.ActivationFunctionType.Exp
    IDENT = mybir.ActivationFunctionType.Identity

    nc = bacc.Bacc("TRN2", target_bir_lowering=False)

    # ---- DRAM I/O (per-core) ----
    xT = nc.dram_tensor("xT", [DIM, S], bf16, kind="ExternalInput")
    # packed: [:,0:128]=[wqA*C1*s|wkB], [:,128:256]=[wkA|wvB],
    #         [:,256:384]=[wvA|wqB*C1*s]
    wall = nc.dram_tensor("wall", [DIM, 384], bf16, kind="ExternalInput")
    # cols: b1=[bqA*C1*s;bkB], b2=[bkA;bvB], b3=[bvA;bqB*C1*s], b4=0
    ball = nc.dram_tensor("ball", [128, 4], f32, kind="ExternalInput")
    wpall = nc.dram_tensor("wpall", [HD, 2 * DIM], bf16, kind="ExternalInput")
    yA = nc.dram_tensor("yA", [DIM, S], bf16, kind="ExternalOutput")
    yB = nc.dram_tensor("yB", [DIM, SH], bf16, kind="ExternalOutput")

    with tile.TileContext(nc) as tc:
        _emit(nc, tc, mybir, make_identity, f32, bf16, i16, EXP, IDENT,
              xT, wall, ball, wpall, yA, yB)

    nc.compile()
    return nc


def _emit(nc, tc, mybir, make_identity, f32, bf16, i16, EXP, IDENT,
          xT, wall, ball, wpall, yA, yB):
    MUL = mybir.AluOpType.mult

    with tc.tile_pool(name="consts", bufs=1) as consts, \
         tc.tile_pool(name="persist", bufs=1) as persist, \
         tc.tile_pool(name="xcp", bufs=3) as xcp, \
         tc.tile_pool(name="vtw", bufs=2) as vtw, \
         tc.tile_pool(name="ptp", bufs=16) as ptp, \
         tc.tile_pool(name="atp", bufs=3) as atp, \
         tc.tile_pool(name="atmp", bufs=3) as atmp, \
         tc.tile_pool(name="normp", bufs=2) as normp, \
         tc.tile_pool(name="outp", bufs=4) as outp, \
         tc.tile_pool(name="ps_gp", bufs=2, space="PSUM") as ps_gp, \
         tc.tile_pool(name="ps_st", bufs=4, space="PSUM") as ps_st, \
         tc.tile_pool(name="ps_o", bufs=1, space="PSUM") as ps_o:

        # ---- constants & weights ----
        identb = consts.tile([128, 128], bf16, tag="ident", name="ident")
        make_identity(nc, identb[:])
        ones32 = consts.tile([128, NKT], bf16, tag="ones32", name="ones32")
        nc.vector.memset(ones32[:], 1.0)
        # match the bit-trick's 2^(5.5/128) scale so both exp paths agree
        bcol = consts.tile([128, 1], f32, tag="bcol", name="bcol")
        nc.vector.memset(bcol[:], float(np.log(2.0) * 5.5 / 128.0))

        # startup critical path: first matmul needs wall[0] + x k-tiles 0:2
        wall_t = [consts.tile([128, 384], bf16, tag=f"wall{k}",
                              name=f"wall{k}") for k in range(KT)]
        nc.sync.dma_start(out=wall_t[0][:], in_=wall[0:128, :])
        xc0 = xcp.tile([128, KT, 1024], bf16, tag="xc", name="xc")
        xTr0 = xT.rearrange("(k p) q -> p k q", p=128)
        nc.sync.dma_start(out=xc0[:, 0:2, 0:512], in_=xTr0[:, 0:2, 0:512])
        nc.sync.dma_start(out=xc0[:, 2:6, 0:512], in_=xTr0[:, 2:6, 0:512])
        ballt = consts.tile([128, 4], f32, tag="ball", name="ball")
        nc.sync.dma_start(out=ballt[:], in_=ball[:])
        for k in range(1, KT):
            nc.sync.dma_start(out=wall_t[k][:],
                              in_=wall[k * 128:(k + 1) * 128, :])
        w1_t = [t[:, 0:128] for t in wall_t]
        w2_t = [t[:, 128:256] for t in wall_t]
        w3_t = [t[:, 256:384] for t in wall_t]
        bias = {f"b{i + 1}": ballt[:, i:i + 1] for i in range(4)}
        wpt = consts.tile([HD, 2 * DIM], bf16, tag="wpall", name="wpall")
        nc.sync.dma_start(out=wpt[:], in_=wpall[:])
        wp_t = {"A": wpt[:, 0:DIM], "B": wpt[:, DIM:2 * DIM]}

        # ---- persistent on-chip tensors ----
        # QAB rows 0:64 = C1-scaled Q^T(A); rows 64:128 = C1-scaled Q^T(B)
        QAB = persist.tile([128, S], bf16, tag="QAB", name="QAB")
        KAB = persist.tile([128, S], bf16, tag="KAB", name="KAB")
        # V_t[:, kt, 0:64]=V_A, col 64=ones, 65:129=V_B, col 129=ones
        V_t = persist.tile([128, NKT, 130], bf16, tag="V", name="V")
        nc.vector.tensor_copy(V_t[:, :, HD], ones32[:])
        nc.vector.tensor_copy(V_t[:, :, 65 + HD], ones32[:])

        # ---- QKV projections: x^T streamed once in [128,1024] chunks ----
        xTr = xT.rearrange("(k p) q -> p k q", p=128)

        def copy_dve(dst, src, b):
            nc.vector.tensor_scalar_add(dst, src, b)

        def copy_act(dst, src, b):
            nc.scalar.activation(dst, src, IDENT, bias=b, scale=1.0)

        def chunk_dma(qc):
            o = qc * 1024
            if qc == 0:
                xc = xc0  # first half already in flight
            else:
                xc = xcp.tile([128, KT, 1024], bf16, tag="xc", name="xc")
                nc.sync.dma_start(out=xc[:, :, 0:512],
                                  in_=xTr[:, :, o:o + 512])
            nc.sync.dma_start(out=xc[:, :, 512:1024],
                              in_=xTr[:, :, o + 512:o + 1024])
            return xc

        def main_chunk(qc, xc):  # qc in 0..3, covers q-cols 1024*qc..+1024
            for h in range(2):
                cs = slice(qc * 1024 + h * 512, qc * 1024 + (h + 1) * 512)
                hs = slice(h * 512, (h + 1) * 512)
                ps1 = ps_gp.tile([128, 512], f32, tag="gp", name="gp1")
                for k in range(KT):
                    nc.tensor.matmul(ps1[:], w1_t[k], xc[:, k, hs],
                                     start=(k == 0), stop=(k == KT - 1))
                copy_dve(QAB[0:HD, cs], ps1[0:HD, :], bias["b1"][0:HD, :])
                copy_act(KAB[HD:128, cs], ps1[HD:128, :],
                         bias["b1"][HD:128, :])
                ps2 = ps_gp.tile([128, 512], f32, tag="gp", name="gp2")
                for k in range(KT):
                    nc.tensor.matmul(ps2[:], w2_t[k], xc[:, k, hs],
                                     start=(k == 0), stop=(k == KT - 1))
                vt = vtw.tile([128, 512], bf16, tag="vt", name="vt")
                copy_act(KAB[0:HD, cs], ps2[0:HD, :], bias["b2"][0:HD, :])
                copy_dve(vt[HD:128, :], ps2[HD:128, :], bias["b2"][HD:128, :])
                ps3 = ps_gp.tile([128, 512], f32, tag="gp", name="gp3")
                for k in range(KT):
                    nc.tensor.matmul(ps3[:], w3_t[k], xc[:, k, hs],
                                     start=(k == 0), stop=(k == KT - 1))
                copy_dve(vt[0:HD, :], ps3[0:HD, :], bias["b3"][0:HD, :])
                copy_act(QAB[HD:128, cs], ps3[HD:128, :],
                         bias["b3"][HD:128, :])
                tp = ps_gp.tile([128, 512], bf16, tag="gp", name="gp4")
                for t4 in range(4):
                    ts_ = slice(t4 * 128, (t4 + 1) * 128)
                    nc.tensor.transpose(tp[:, ts_], vt[:, ts_], identb[:])
                kt0 = qc * 8 + h * 4
                tpr = tp[:].rearrange("p (t c) -> p t c", t=4)
                nc.scalar.copy(V_t[:, kt0:kt0 + 4, 0:HD], tpr[:, :, 0:HD])
                nc.vector.tensor_copy(V_t[:, kt0:kt0 + 4, 65:65 + HD],
                                      tpr[:, :, HD:128])

        # ---- attention super-sweeps + fused projection ----
        pid = nc.sync.partition_id()
        QBloc = persist.tile([128, SH], bf16, tag="QBloc", name="QBloc")
        vo = {"A": 0, "B": 65}
        rowsl = {"A": slice(0, HD), "B": slice(HD, 128)}
        tpos = {"A": (0, 0), "B": (64, 0)}
        sweeps = [("A", 0, "A", 1), ("A", 2, "B", 0), ("A", 3, "B", 1),
                  ("A", 4, "B", 2), ("A", 5, "B", 3), ("A", 6, "A", 7)]

        hold = {"pending": [], "pvq": []}

        def finish_units(jobs, out_ps):
            # normalize NOW (frees out_ps for the next sweep), defer the rest
            rc = normp.tile([128, 8], f32, tag="rc", name="rc")
            nc.vector.reciprocal(rc[:], out_ps[:, :, HD])
            # at is hd-padded to 128 so the XBAR transpose below sees a
            # 128-multiple free dim; cols 64:128 are never-read garbage
            at = atp.tile([128, 8, 128], bf16, tag="at", name="at")
            nc.vector.tensor_tensor(
                at[:, :, 0:HD], out_ps[:, :, 0:HD],
                rc[:].rearrange("p (a b) -> p a b", b=1).broadcast_to(
                    [128, 8, HD]),
                MUL)
            # (out_ps cols 65:128 are dead padding - keeps PV chunks off
            # PSUM bank boundaries)

            units = []
            cells = [{} for _ in jobs]

            def mk_tr(i, j):
                def u():
                    atT = ps_gp.tile([HD, 4, 128], bf16, tag="gp", name="atT")
                    for c in range(4):
                        nc.tensor.transpose(atT[:, c, :],
                                            at[:, i * 4 + c, 0:HD],
                                            identb[:])
                    cells[i]["atT"] = atT
                return u

            def mk_atm(i, j):
                def u():
                    atm = atmp.tile([HD, 512], bf16, tag="atm", name="atm")
                    nc.scalar.copy(
                        atm[:].rearrange("p (t c) -> p t c", t=4),
                        cells[i]["atT"][:])
                    cp = outp.tile([128, KT, 512], bf16, tag="cp", name="cp")
                    cells[i]["atm"] = atm
                    cells[i]["cp"] = cp
                return u

            def mk_proj(i, j, m):
                def u():
                    atm, cp = cells[i]["atm"], cells[i]["cp"]
                    pp = ps_gp.tile([128, 512], f32, tag="gp", name="pp")
                    nc.tensor.matmul(pp[:], wp_t[j][:, m * 128:(m + 1) * 128],
                                     atm[:], start=True, stop=True)
                    if m % 2 == 1 or (m == 2 and i == 0):
                        nc.scalar.copy(cp[:, m, :], pp[:])
                    else:
                        nc.vector.tensor_copy(cp[:, m, :], pp[:])
                return u

            def mk_dma(i, j, q, m0):
                def u():
                    ydram = yA if j == "A" else yB
                    ydr = ydram.rearrange("(m p) q -> p m q", p=128)
                    nc.sync.dma_start(
                        out=ydr[:, m0:m0 + 2, q * 512:(q + 1) * 512],
                        in_=cells[i]["cp"][:, m0:m0 + 2, :])
                return u

            for i, (j, q) in enumerate(jobs):
                units.append(mk_tr(i, j))
                units.append(mk_atm(i, j))
            for m in range(KT):
                for i, (j, q) in enumerate(jobs):
                    units.append(mk_proj(i, j, m))
                if m % 2 == 1 and m < KT - 1:
                    for i, (j, q) in enumerate(jobs):
                        units.append(mk_dma(i, j, q, m - 1))
            for i, (j, q) in enumerate(jobs):
                units.append(mk_dma(i, j, q, KT - 2))
            return units

        def begin_sweep(jobs):
            out_ps = ps_o.tile([128, 8, 128], f32, tag="out", name="out")
            return {"jobs": jobs, "out": out_ps, "kt": 0}

        def emit_kts(ss, n):
            jobs, out_ps = ss["jobs"], ss["out"]
            for _ in range(n):
                kt = ss["kt"]
                sts = []
                for i, (j, q) in enumerate(jobs):
                    st = ps_st.tile([128, 512], f32, tag="st",
                                    name=f"st{i}")
                    qsrc = QBloc if j == "B" else QAB
                    nc.tensor.matmul(st[:],
                                     KAB[rowsl[j], kt * 128:(kt + 1) * 128],
                                     qsrc[rowsl[j], q * 512:(q + 1) * 512],
                                     start=True, stop=True,
                                     tile_position=tpos[j])
                    sts.append(st)
                pt = ptp.tile([128, 1024], bf16, tag="pt", name="pt")
                # job0 -> scalar engine (true exp); job1 -> DVE bit trick
                nc.scalar.activation(pt[:, 0:512], sts[0][:],
                                     EXP, bias=bcol[:], scale=1.0 / C1)
                nc.vector.tensor_scalar_add(
                    pt[:, 512:1024].bitcast(i16), sts[1][:], C2V)

                def pv(kt=kt, pt=pt):
                    # out_ps slots share 2KB PSUM zero-regions (4 slots per
                    # bank): only the first slot of each bank may raise
                    # start_tensor_calc (it arms/zeroes the whole region) and
                    # only the last slot stops it.
                    for c in range(8):
                        j = jobs[c // 4][0]
                        nc.tensor.matmul(out_ps[:, c, 0:65],
                                         pt[:, c * 128:(c + 1) * 128],
                                         V_t[:, kt, vo[j]:vo[j] + 65],
                                         start=(kt == 0 and c % 4 == 0),
                                         stop=(kt == NKT - 1 and c % 4 == 3),
                                         skip_group_check=True)
                hold["pvq"].append(pv)
                while len(hold["pvq"]) > 12:
                    hold["pvq"].pop(0)()
                ss["kt"] += 1
                if hold["pending"]:
                    hold["pending"].pop(0)()

        def end_sweep(ss):
            # defer normalize + unit work behind the in-flight PVs so the
            # next sweep's QK/convert stream overlaps this sweep's tail
            def norm_then_units(ss=ss):
                hold["pending"].extend(
                    finish_units(ss["jobs"], ss["out"]))
            hold["pvq"].append(norm_then_units)

        # fused QKV + sweep 0 (x chunk DMAs prefetched one chunk ahead)
        ss0 = begin_sweep([(sweeps[0][0], sweeps[0][1]),
                           (sweeps[0][2], sweeps[0][3])])
        xcs = [chunk_dma(0), chunk_dma(1)]
        main_chunk(0, xcs[0])
        xcs.append(chunk_dma(2))
        emit_kts(ss0, 8)
        main_chunk(1, xcs[1])
        xcs.append(chunk_dma(3))
        emit_kts(ss0, 8)
        main_chunk(2, xcs[2])
        emit_kts(ss0, 8)
        main_chunk(3, xcs[3])
        if SIM_STATIC:
            nc.sync.dma_start(out=QBloc[HD:128, :], in_=QAB[HD:128, 0:SH])
        else:
            with tc.If((pid & 1) < 1) as cmp:
                nc.sync.dma_start(out=QBloc[HD:128, :], in_=QAB[HD:128, 0:SH])
            with cmp.Else():
                nc.sync.dma_start(out=QBloc[HD:128, :],
                                  in_=QAB[HD:128, SH:2 * SH])
        emit_kts(ss0, 8)
        end_sweep(ss0)

        for si, (j0, q0, j1, q1) in enumerate(sweeps[1:]):
            ss = begin_sweep([(j0, q0), (j1, q1)])
            emit_kts(ss, NKT)
            end_sweep(ss)
        while hold["pvq"]:
            hold["pvq"].pop(0)()
        while hold["pending"]:
            hold["pending"].pop(0)()


def _get_nc():
    if "nc" not in _CACHE:
        _CACHE["nc"] = _build_nc()
    return _CACHE["nc"]


def kernel(x, w_qkv, b_qkv, w_proj, b_proj):
    from concourse.bass_utils import run_bass_kernel_spmd

    BF = ml_dtypes.bfloat16
    x = np.asarray(x, dtype=np.float32)
    w_qkv = np.asarray(w_qkv, dtype=np.float32)
    b_qkv = np.asarray(b_qkv, dtype=np.float32)
    w_proj = np.asarray(w_proj, dtype=np.float32)
    b_proj = np.asarray(b_proj, dtype=np.float32)

    B = x.shape[0]
    xT = np.ascontiguousarray(x[0].T).astype(BF)  # [768, 4096]
    QS = SCALE * C1

    def wcol(block, h):
        o = block * DIM + h * HD
        return w_qkv[:, o:o + HD]

    def bcol_(block, h):
        o = block * DIM + h * HD
        return b_qkv[o:o + HD]

    in_maps = []
    meta = []
    z64 = np.zeros(HD, dtype=np.float32)
    for c in range(NCORES):
        hA, hB, qh = c, 8 + c // 2, c % 2
        m = {
            "xT": xT,
            "wall": np.concatenate(
                [wcol(0, hA) * QS, wcol(1, hB), wcol(1, hA), wcol(2, hB),
                 wcol(2, hA), wcol(0, hB) * QS], axis=1).astype(BF),
            "ball": np.stack(
                [np.concatenate([bcol_(0, hA) * QS, bcol_(1, hB)]),
                 np.concatenate([bcol_(1, hA), bcol_(2, hB)]),
                 np.concatenate([bcol_(2, hA), bcol_(0, hB) * QS]),
                 np.concatenate([z64, z64])], axis=1).astype(np.float32),
            "wpall": np.concatenate(
                [w_proj[hA * HD:(hA + 1) * HD, :],
                 w_proj[hB * HD:(hB + 1) * HD, :]], axis=1).astype(BF),
        }
        in_maps.append({k: np.ascontiguousarray(v) for k, v in m.items()})
        meta.append(qh)

    nc = _get_nc()
    res = run_bass_kernel_spmd(nc, in_maps, core_ids=list(range(NCORES)))

    Y = np.zeros((DIM, S), dtype=np.float64)
    for c in range(NCORES):
        Y += res.results[c]["yA"].astype(np.float64)
        qh = meta[c]
        Y[:, qh * SH:(qh + 1) * SH] += res.results[c]["yB"].astype(np.float64)
    out = (Y.T + b_proj.astype(np.float64)).astype(np.float32)
    return out.reshape(B, S, DIM)


# revision 58
# speedup vs baseline: 1.0004x; 1.0004x over previous
"""Multi-head self-attention (B=1, S=4096, DIM=768, H=12) on 8 Trainium2
NeuronCores.

Sharding: tensor-parallel over heads. Core c computes
  - full attention for head hA = c            (heads 0..7, all 4096 queries)
  - half attention for head hB = 8 + c//2     (heads 8..11, query half c%2)
Each core computes its own K/V projections for its two heads from x^T
streamed through SBUF once, runs attention fully on-chip, applies its heads'
slice of the output projection, and returns transposed partial projections
(bf16). The host sums per-core partials (the tensor-parallel all-reduce),
adds b_proj, and transposes back.

Fast path vs the fp32 baseline:
  - All matmuls in bf16 (x, QKV weights, Q/K/V, pt, proj weights).
  - exp(score) is split across two engines BY JOB: the scalar engine
    computes true exp for the sweep's first job; the vector engine handles
    the second job with a Schraudolph bit trick: Q is pre-scaled by
    C1 = 128/ln2 so the QK matmul emits scores in bf16-bits domain, then one
    tensor_scalar_add(+16261.5) with int16 output writes exp(s)*1.03 bit
    patterns directly into the bf16 pt tile (rel err ~3%, cancels in
    softmax's numerator/denominator except per-weight ripple). Each job's
    scores live in their own 1-bank PSUM tile (4-deep rotation) and PV
    consumption is deferred two k-tiles so neither engine's latency sits on
    the PE critical path.
  - PV is "flipped": stationary = pt q-chunk [128,128], moving = [V|1]
    [128,65], so each matmul streams only 65 output columns (the cost model
    charges output free size). Output lands q-on-partitions with the softmax
    denominator in column 64, so normalization is a per-partition reciprocal
    + broadcast multiply (no partition broadcast), then a cheap PE transpose
    restores [hd, q] for the output projection.
"""

import numpy as np
import ml_dtypes

DIM = 768
HEADS = 12
HD = 64
SCALE = HD ** (-0.5)
S = 4096
SH = 2048
NCORES = 8
KT = DIM // 128   # 6 k-tiles over the 768 contraction dim
NKT = S // 128    # 32 k-tiles over the 4096 sequence dim

C1 = 128.0 / np.log(2.0)   # schraudolph scale, folded into Q weights
C2V = 16256.0 + 5.5        # bf16 bits of 1.0 + centering, added at convert

_CACHE: dict = {}
SIM_STATIC = False  # profile scripts set True: TimelineSim can't eval branches


def _build_nc():
    import concourse.bacc as bacc
    import concourse.tile as tile
    from concourse import mybir
    from concourse.masks import make_identity

    f32 = mybir.dt.float32
    bf16 = mybir.dt.bfloat16
    i16 = mybir.dt.int16
    EXP = mybir.ActivationFunctionType.Exp
    IDENT = mybir.ActivationFunctionType.Identity

    nc = bacc.Bacc("TRN2", target_bir_lowering=False)

    # ---- DRAM I/O (per-core) ----
    xT = nc.dram_tensor("xT", [DIM, S], bf16, kind="ExternalInput")
    # packed: [:,0:128]=[wqA*C1*s|wkB], [:,128:256]=[wkA|wvB],
    #         [:,256:384]=[wvA|wqB*C1*s]
    wall = nc.dram_tensor("wall", [DIM, 384], bf16, kind="ExternalInput")
    # cols: b1=[bqA*C1*s;bkB], b2=[bkA;bvB], b3=[bvA;bqB*C1*s], b4=0
    ball = nc.dram_tensor("ball", [128, 4], f32, kind="ExternalInput")
    wpall = nc.dram_tensor("wpall", [HD, 2 * DIM], bf16, kind="ExternalInput")
    yA = nc.dram_tensor("yA", [DIM, S], bf16, kind="ExternalOutput")
    yB = nc.dram_tensor("yB", [DIM, SH], bf16, kind="ExternalOutput")

    with tile.TileContext(nc) as tc:
        _emit(nc, tc, mybir, make_identity, f32, bf16, i16, EXP, IDENT,
              xT, wall, ball, wpall, yA, yB)

    nc.compile()
    return nc


def _emit(nc, tc, mybir, make_identity, f32, bf16, i16, EXP, IDENT,
          xT, wall, ball, wpall, yA, yB):
    MUL = mybir.AluOpType.mult

    with tc.tile_pool(name="consts", bufs=1) as consts, \
         tc.tile_pool(name="persist", bufs=1) as persist, \
         tc.tile_pool(name="xcp", bufs=3) as xcp, \
         tc.tile_pool(name="vtw", bufs=2) as vtw, \
         tc.tile_pool(name="ptp", bufs=20) as ptp, \
         tc.tile_pool(name="atp", bufs=3) as atp, \
         tc.tile_pool(name="atmp", bufs=4) as atmp, \
         tc.tile_pool(name="normp", bufs=3) as normp, \
         tc.tile_pool(name="outp", bufs=5) as outp, \
         tc.tile_pool(name="ps_gp", bufs=2, space="PSUM") as ps_gp, \
         tc.tile_pool(name="ps_st", bufs=4, space="PSUM") as ps_st, \
         tc.tile_pool(name="ps_o", bufs=1, space="PSUM") as ps_o:

        # ---- constants & weights ----
        identb = consts.tile([128, 128], bf16, tag="ident", name="ident")
        make_identity(nc, identb[:])
        ones32 = consts.tile([128, NKT], bf16, tag="ones32", name="ones32")
        nc.vector.memset(ones32[:], 1.0)
        # match the bit-trick's 2^(5.5/128) scale so both exp paths agree
        bcol = consts.tile([128, 1], f32, tag="bcol", name="bcol")
        nc.vector.memset(bcol[:], float(np.log(2.0) * 5.5 / 128.0))

        # startup critical path: first matmul needs wall[0] + x k-tiles 0:2
        wall_t = [consts.tile([128, 384], bf16, tag=f"wall{k}",
                              name=f"wall{k}") for k in range(KT)]
        nc.sync.dma_start(out=wall_t[0][:], in_=wall[0:128, :])
        xc0 = xcp.tile([128, KT, 1024], bf16, tag="xc", name="xc")
        xTr0 = xT.rearrange("(k p) q -> p k q", p=128)
        nc.sync.dma_start(out=xc0[:, 0:2, 0:512], in_=xTr0[:, 0:2, 0:512])
        nc.sync.dma_start(out=xc0[:, 2:6, 0:512], in_=xTr0[:, 2:6, 0:512])
        ballt = consts.tile([128, 4], f32, tag="ball", name="ball")
        nc.sync.dma_start(out=ballt[:], in_=ball[:])
        for k in range(1, KT):
            nc.sync.dma_start(out=wall_t[k][:],
                              in_=wall[k * 128:(k + 1) * 128, :])
        w1_t = [t[:, 0:128] for t in wall_t]
        w2_t = [t[:, 128:256] for t in wall_t]
        w3_t = [t[:, 256:384] for t in wall_t]
        bias = {f"b{i + 1}": ballt[:, i:i + 1] for i in range(4)}
        wpt = consts.tile([HD, 2 * DIM], bf16, tag="wpall", name="wpall")
        nc.sync.dma_start(out=wpt[:], in_=wpall[:])
        wp_t = {"A": wpt[:, 0:DIM], "B": wpt[:, DIM:2 * DIM]}

        # ---- persistent on-chip tensors ----
        # QAB rows 0:64 = C1-scaled Q^T(A); rows 64:128 = C1-scaled Q^T(B)
        QAB = persist.tile([128, S], bf16, tag="QAB", name="QAB")
        KAB = persist.tile([128, S], bf16, tag="KAB", name="KAB")
        # V_t[:, kt, 0:64]=V_A, col 64=ones, 65:129=V_B, col 129=ones
        V_t = persist.tile([128, NKT, 130], bf16, tag="V", name="V")
        nc.vector.tensor_copy(V_t[:, :, HD], ones32[:])
        nc.vector.tensor_copy(V_t[:, :, 65 + HD], ones32[:])

        # ---- QKV projections: x^T streamed once in [128,1024] chunks ----
        xTr = xT.rearrange("(k p) q -> p k q", p=128)

        def copy_dve(dst, src, b):
            nc.vector.tensor_scalar_add(dst, src, b)

        def copy_act(dst, src, b):
            nc.scalar.activation(dst, src, IDENT, bias=b, scale=1.0)

        def chunk_dma(qc):
            o = qc * 1024
            if qc == 0:
                xc = xc0  # first half already in flight
            else:
                xc = xcp.tile([128, KT, 1024], bf16, tag="xc", name="xc")
                nc.sync.dma_start(out=xc[:, :, 0:512],
                                  in_=xTr[:, :, o:o + 512])
            nc.sync.dma_start(out=xc[:, :, 512:1024],
                              in_=xTr[:, :, o + 512:o + 1024])
            return xc

        def main_chunk(qc, xc):  # qc in 0..3, covers q-cols 1024*qc..+1024
            for h in range(2):
                cs = slice(qc * 1024 + h * 512, qc * 1024 + (h + 1) * 512)
                hs = slice(h * 512, (h + 1) * 512)
                ps1 = ps_gp.tile([128, 512], f32, tag="gp", name="gp1")
                for k in range(KT):
                    nc.tensor.matmul(ps1[:], w1_t[k], xc[:, k, hs],
                                     start=(k == 0), stop=(k == KT - 1))
                copy_dve(QAB[0:HD, cs], ps1[0:HD, :], bias["b1"][0:HD, :])
                copy_act(KAB[HD:128, cs], ps1[HD:128, :],
                         bias["b1"][HD:128, :])
                ps2 = ps_gp.tile([128, 512], f32, tag="gp", name="gp2")
                for k in range(KT):
                    nc.tensor.matmul(ps2[:], w2_t[k], xc[:, k, hs],
                                     start=(k == 0), stop=(k == KT - 1))
                vt = vtw.tile([128, 512], bf16, tag="vt", name="vt")
                copy_act(KAB[0:HD, cs], ps2[0:HD, :], bias["b2"][0:HD, :])
                copy_dve(vt[HD:128, :], ps2[HD:128, :], bias["b2"][HD:128, :])
                ps3 = ps_gp.tile([128, 512], f32, tag="gp", name="gp3")
                for k in range(KT):
                    nc.tensor.matmul(ps3[:], w3_t[k], xc[:, k, hs],
                                     start=(k == 0), stop=(k == KT - 1))
                copy_dve(vt[0:HD, :], ps3[0:HD, :], bias["b3"][0:HD, :])
                copy_act(QAB[HD:128, cs], ps3[HD:128, :],
                         bias["b3"][HD:128, :])
                tp = ps_gp.tile([128, 512], bf16, tag="gp", name="gp4")
                for t4 in range(4):
                    ts_ = slice(t4 * 128, (t4 + 1) * 128)
                    nc.tensor.transpose(tp[:, ts_], vt[:, ts_], identb[:])
                kt0 = qc * 8 + h * 4
                tpr = tp[:].rearrange("p (t c) -> p t c", t=4)
                nc.scalar.copy(V_t[:, kt0:kt0 + 4, 0:HD], tpr[:, :, 0:HD])
                nc.vector.tensor_copy(V_t[:, kt0:kt0 + 4, 65:65 + HD],
                                      tpr[:, :, HD:128])

        # ---- attention super-sweeps + fused projection ----
        pid = nc.sync.partition_id()
        QBloc = persist.tile([128, SH], bf16, tag="QBloc", name="QBloc")
        vo = {"A": 0, "B": 65}
        rowsl = {"A": slice(0, HD), "B": slice(HD, 128)}
        tpos = {"A": (0, 0), "B": (64, 0)}
        sweeps = [("A", 0, "A", 1), ("A", 2, "B", 0), ("A", 3, "B", 1),
                  ("A", 4, "B", 2), ("A", 5, "B", 3), ("A", 6, "A", 7)]

        hold = {"pending": [], "pvq": []}

        def finish_units(jobs, out_ps):
            # normalize NOW (frees out_ps for the next sweep), defer the rest
            rc = normp.tile([128, 8], f32, tag="rc", name="rc")
            nc.vector.reciprocal(rc[:], out_ps[:, :, HD])
            # at is hd-padded to 128 so the XBAR transpose below sees a
            # 128-multiple free dim; cols 64:128 are never-read garbage
            at = atp.tile([128, 8, 128], bf16, tag="at", name="at")
            nc.vector.tensor_tensor(
                at[:, :, 0:HD], out_ps[:, :, 0:HD],
                rc[:].rearrange("p (a b) -> p a b", b=1).broadcast_to(
                    [128, 8, HD]),
                MUL)
            # (out_ps cols 65:128 are dead padding - keeps PV chunks off
            # PSUM bank boundaries)

            units = []
            cells = [{} for _ in jobs]

            def mk_tr(i, j):
                def u():
                    atT = ps_gp.tile([HD, 4, 128], bf16, tag="gp", name="atT")
                    for c in range(4):
                        nc.tensor.transpose(atT[:, c, :],
                                            at[:, i * 4 + c, 0:HD],
                                            identb[:])
                    cells[i]["atT"] = atT
                return u

            def mk_atm(i, j):
                def u():
                    atm = atmp.tile([HD, 512], bf16, tag="atm", name="atm")
                    nc.scalar.copy(
                        atm[:].rearrange("p (t c) -> p t c", t=4),
                        cells[i]["atT"][:])
                    cp = outp.tile([128, KT, 512], bf16, tag="cp", name="cp")
                    cells[i]["atm"] = atm
                    cells[i]["cp"] = cp
                return u

            def mk_proj(i, j, m):
                def u():
                    atm, cp = cells[i]["atm"], cells[i]["cp"]
                    pp = ps_gp.tile([128, 512], f32, tag="gp", name="pp")
                    nc.tensor.matmul(pp[:], wp_t[j][:, m * 128:(m + 1) * 128],
                                     atm[:], start=True, stop=True)
                    if m % 2 == 1 or (m == 2 and i == 0):
                        nc.scalar.copy(cp[:, m, :], pp[:])
                    else:
                        nc.vector.tensor_copy(cp[:, m, :], pp[:])
                return u

            def mk_dma(i, j, q, m0):
                def u():
                    ydram = yA if j == "A" else yB
                    ydr = ydram.rearrange("(m p) q -> p m q", p=128)
                    nc.sync.dma_start(
                        out=ydr[:, m0:m0 + 2, q * 512:(q + 1) * 512],
                        in_=cells[i]["cp"][:, m0:m0 + 2, :])
                return u

            for i, (j, q) in enumerate(jobs):
                units.append(mk_tr(i, j))
                units.append(mk_atm(i, j))
            for m in range(KT):
                for i, (j, q) in enumerate(jobs):
                    units.append(mk_proj(i, j, m))
                if m % 2 == 1 and m < KT - 1:
                    for i, (j, q) in enumerate(jobs):
                        units.append(mk_dma(i, j, q, m - 1))
            for i, (j, q) in enumerate(jobs):
                units.append(mk_dma(i, j, q, KT - 2))
            return units

        def begin_sweep(jobs):
            out_ps = ps_o.tile([128, 8, 128], f32, tag="out", name="out")
            return {"jobs": jobs, "out": out_ps, "kt": 0}

        def emit_kts(ss, n):
            jobs, out_ps = ss["jobs"], ss["out"]
            for _ in range(n):
                kt = ss["kt"]
                sts = []
                for i, (j, q) in enumerate(jobs):
                    st = ps_st.tile([128, 512], f32, tag="st",
                                    name=f"st{i}")
                    qsrc = QBloc if j == "B" else QAB
                    nc.tensor.matmul(st[:],
                                     KAB[rowsl[j], kt * 128:(kt + 1) * 128],
                                     qsrc[rowsl[j], q * 512:(q + 1) * 512],
                                     start=True, stop=True,
                                     tile_position=tpos[j])
                    sts.append(st)
                pt = ptp.tile([128, 1024], bf16, tag="pt", name="pt")
                # job0 -> scalar engine (true exp); job1 -> DVE bit trick
                nc.scalar.activation(pt[:, 0:512], sts[0][:],
                                     EXP, bias=bcol[:], scale=1.0 / C1)
                nc.vector.tensor_scalar_add(
                    pt[:, 512:1024].bitcast(i16), sts[1][:], C2V)

                def pv(kt=kt, pt=pt):
                    # out_ps slots share 2KB PSUM zero-regions (4 slots per
                    # bank): only the first slot of each bank may raise
                    # start_tensor_calc (it arms/zeroes the whole region) and
                    # only the last slot stops it.
                    for c in range(8):
                        j = jobs[c // 4][0]
                        nc.tensor.matmul(out_ps[:, c, 0:65],
                                         pt[:, c * 128:(c + 1) * 128],
                                         V_t[:, kt, vo[j]:vo[j] + 65],
                                         start=(kt == 0 and c % 4 == 0),
                                         stop=(kt == NKT - 1 and c % 4 == 3),
                                         skip_group_check=True)
                hold["pvq"].append(pv)
                while len(hold["pvq"]) > 12:
                    hold["pvq"].pop(0)()
                ss["kt"] += 1
                if hold["pending"]:
                    hold["pending"].pop(0)()

        def end_sweep(ss):
            # defer normalize + unit work behind the in-flight PVs so the
            # next sweep's QK/convert stream overlaps this sweep's tail
            def norm_then_units(ss=ss):
                hold["pending"].extend(
                    finish_units(ss["jobs"], ss["out"]))
            hold["pvq"].append(norm_then_units)

        # fused QKV + sweep 0 (x chunk DMAs prefetched one chunk ahead)
        ss0 = begin_sweep([(sweeps[0][0], sweeps[0][1]),
                           (sweeps[0][2], sweeps[0][3])])
        xcs = [chunk_dma(0), chunk_dma(1)]
        main_chunk(0, xcs[0])
        xcs.append(chunk_dma(2))
        emit_kts(ss0, 8)
        main_chunk(1, xcs[1])
        xcs.append(chunk_dma(3))
        emit_kts(ss0, 8)
        main_chunk(2, xcs[2])
        emit_kts(ss0, 8)
        main_chunk(3, xcs[3])
        if SIM_STATIC:
            nc.sync.dma_start(out=QBloc[HD:128, :], in_=QAB[HD:128, 0:SH])
        else:
            with tc.If((pid & 1) < 1) as cmp:
                nc.sync.dma_start(out=QBloc[HD:128, :], in_=QAB[HD:128, 0:SH])
            with cmp.Else():
                nc.sync.dma_start(out=QBloc[HD:128, :],
                                  in_=QAB[HD:128, SH:2 * SH])
        emit_kts(ss0, 8)
        end_sweep(ss0)

        for si, (j0, q0, j1, q1) in enumerate(sweeps[1:]):
            ss = begin_sweep([(j0, q0), (j1, q1)])
            emit_kts(ss, NKT)
            end_sweep(ss)
        while hold["pvq"]:
            hold["pvq"].pop(0)()
        while hold["pending"]:
            hold["pending"].pop(0)()


def _get_nc():
    if "nc" not in _CACHE:
        _CACHE["nc"] = _build_nc()
    return _CACHE["nc"]


def kernel(x, w_qkv, b_qkv, w_proj, b_proj):
    from concourse.bass_utils import run_bass_kernel_spmd

    BF = ml_dtypes.bfloat16
    x = np.asarray(x, dtype=np.float32)
    w_qkv = np.asarray(w_qkv, dtype=np.float32)
    b_qkv = np.asarray(b_qkv, dtype=np.float32)
    w_proj = np.asarray(w_proj, dtype=np.float32)
    b_proj = np.asarray(b_proj, dtype=np.float32)

    B = x.shape[0]
    xT = np.ascontiguousarray(x[0].T).astype(BF)  # [768, 4096]
    QS = SCALE * C1

    def wcol(block, h):
        o = block * DIM + h * HD
        return w_qkv[:, o:o + HD]

    def bcol_(block, h):
        o = block * DIM + h * HD
        return b_qkv[o:o + HD]

    in_maps = []
    meta = []
    z64 = np.zeros(HD, dtype=np.float32)
    for c in range(NCORES):
        hA, hB, qh = c, 8 + c // 2, c % 2
        m = {
            "xT": xT,
            "wall": np.concatenate(
                [wcol(0, hA) * QS, wcol(1, hB), wcol(1, hA), wcol(2, hB),
                 wcol(2, hA), wcol(0, hB) * QS], axis=1).astype(BF),
            "ball": np.stack(
                [np.concatenate([bcol_(0, hA) * QS, bcol_(1, hB)]),
                 np.concatenate([bcol_(1, hA), bcol_(2, hB)]),
                 np.concatenate([bcol_(2, hA), bcol_(0, hB) * QS]),
                 np.concatenate([z64, z64])], axis=1).astype(np.float32),
            "wpall": np.concatenate(
                [w_proj[hA * HD:(hA + 1) * HD, :],
                 w_proj[hB * HD:(hB + 1) * HD, :]], axis=1).astype(BF),
        }
        in_maps.append({k: np.ascontiguousarray(v) for k, v in m.items()})
        meta.append(qh)

    nc = _get_nc()
    res = run_bass_kernel_spmd(nc, in_maps, core_ids=list(range(NCORES)))

    Y = np.zeros((DIM, S), dtype=np.float64)
    for c in range(NCORES):
        Y += res.results[c]["yA"].astype(np.float64)
        qh = meta[c]
        Y[:, qh * SH:(qh + 1) * SH] += res.results[c]["yB"].astype(np.float64)
    out = (Y.T + b_proj.astype(np.float64)).astype(np.float32)
    return out.reshape(B, S, DIM)


# revision 60
# speedup vs baseline: 1.0012x; 1.0009x over previous
"""Multi-head self-attention (B=1, S=4096, DIM=768, H=12) on 8 Trainium2
NeuronCores.

Sharding: tensor-parallel over heads. Core c computes
  - full attention for head hA = c            (heads 0..7, all 4096 queries)
  - half attention for head hB = 8 + c//2     (heads 8..11, query half c%2)
Each core computes its own K/V projections for its two heads from x^T
streamed through SBUF once, runs attention fully on-chip, applies its heads'
slice of the output projection, and returns transposed partial projections
(bf16). The host sums per-core partials (the tensor-parallel all-reduce),
adds b_proj, and transposes back.

Fast path vs the fp32 baseline:
  - All matmuls in bf16 (x, QKV weights, Q/K/V, pt, proj weights).
  - exp(score) is split across two engines BY JOB: the scalar engine
    computes true exp for the sweep's first job; the vector engine handles
    the second job with a Schraudolph bit trick: Q is pre-scaled by
    C1 = 128/ln2 so the QK matmul emits scores in bf16-bits domain, then one
    tensor_scalar_add(+16261.5) with int16 output writes exp(s)*1.03 bit
    patterns directly into the bf16 pt tile (rel err ~3%, cancels in
    softmax's numerator/denominator except per-weight ripple). Each job's
    scores live in their own 1-bank PSUM tile (4-deep rotation) and PV
    consumption is deferred two k-tiles so neither engine's latency sits on
    the PE critical path.
  - PV is "flipped": stationary = pt q-chunk [128,128], moving = [V|1]
    [128,65], so each matmul streams only 65 output columns (the cost model
    charges output free size). Output lands q-on-partitions with the softmax
    denominator in column 64, so normalization is a per-partition reciprocal
    + broadcast multiply (no partition broadcast), then a cheap PE transpose
    restores [hd, q] for the output projection.
"""

import numpy as np
import ml_dtypes

DIM = 768
HEADS = 12
HD = 64
SCALE = HD ** (-0.5)
S = 4096
SH = 2048
NCORES = 8
KT = DIM // 128   # 6 k-tiles over the 768 contraction dim
NKT = S // 128    # 32 k-tiles over the 4096 sequence dim

C1 = 128.0 / np.log(2.0)   # schraudolph scale, folded into Q weights
C2V = 16256.0 + 5.5        # bf16 bits of 1.0 + centering, added at convert

_CACHE: dict = {}
SIM_STATIC = False  # profile scripts set True: TimelineSim can't eval branches


def _build_nc():
    import concourse.bacc as bacc
    import concourse.tile as tile
    from concourse import mybir
    from concourse.masks import make_identity

    f32 = mybir.dt.float32
    bf16 = mybir.dt.bfloat16
    i16 = mybir.dt.int16
    EXP = mybir.ActivationFunctionType.Exp
    IDENT = mybir.ActivationFunctionType.Identity

    nc = bacc.Bacc("TRN2", target_bir_lowering=False)

    # ---- DRAM I/O (per-core) ----
    xT = nc.dram_tensor("xT", [DIM, S], bf16, kind="ExternalInput")
    # packed: [:,0:128]=[wqA*C1*s|wkB], [:,128:256]=[wkA|wvB],
    #         [:,256:384]=[wvA|wqB*C1*s]
    wall = nc.dram_tensor("wall", [DIM, 384], bf16, kind="ExternalInput")
    # cols: b1=[bqA*C1*s;bkB], b2=[bkA;bvB], b3=[bvA;bqB*C1*s], b4=0
    ball = nc.dram_tensor("ball", [128, 4], f32, kind="ExternalInput")
    wpall = nc.dram_tensor("wpall", [HD, 2 * DIM], bf16, kind="ExternalInput")
    yA = nc.dram_tensor("yA", [DIM, S], bf16, kind="ExternalOutput")
    yB = nc.dram_tensor("yB", [DIM, SH], bf16, kind="ExternalOutput")

    with tile.TileContext(nc) as tc:
        _emit(nc, tc, mybir, make_identity, f32, bf16, i16, EXP, IDENT,
              xT, wall, ball, wpall, yA, yB)

    nc.compile()
    return nc


def _emit(nc, tc, mybir, make_identity, f32, bf16, i16, EXP, IDENT,
          xT, wall, ball, wpall, yA, yB):
    MUL = mybir.AluOpType.mult

    with tc.tile_pool(name="consts", bufs=1) as consts, \
         tc.tile_pool(name="persist", bufs=1) as persist, \
         tc.tile_pool(name="xcp", bufs=3) as xcp, \
         tc.tile_pool(name="vtw", bufs=2) as vtw, \
         tc.tile_pool(name="ptp", bufs=20) as ptp, \
         tc.tile_pool(name="atp", bufs=3) as atp, \
         tc.tile_pool(name="atmp", bufs=4) as atmp, \
         tc.tile_pool(name="normp", bufs=3) as normp, \
         tc.tile_pool(name="outp", bufs=5) as outp, \
         tc.tile_pool(name="ps_gp", bufs=2, space="PSUM") as ps_gp, \
         tc.tile_pool(name="ps_st", bufs=4, space="PSUM") as ps_st, \
         tc.tile_pool(name="ps_o", bufs=1, space="PSUM") as ps_o:

        # ---- constants & weights ----
        identb = consts.tile([128, 128], bf16, tag="ident", name="ident")
        make_identity(nc, identb[:])
        ones32 = consts.tile([128, NKT], bf16, tag="ones32", name="ones32")
        nc.vector.memset(ones32[:], 1.0)
        # match the bit-trick's 2^(5.5/128) scale so both exp paths agree
        bcol = consts.tile([128, 1], f32, tag="bcol", name="bcol")
        nc.vector.memset(bcol[:], float(np.log(2.0) * 5.5 / 128.0))

        # startup critical path: first matmul needs wall[0] + x k-tiles 0:2
        wall_t = [consts.tile([128, 384], bf16, tag=f"wall{k}",
                              name=f"wall{k}") for k in range(KT)]
        nc.sync.dma_start(out=wall_t[0][:], in_=wall[0:128, :])
        xc0 = xcp.tile([128, KT, 1024], bf16, tag="xc", name="xc")
        xTr0 = xT.rearrange("(k p) q -> p k q", p=128)
        nc.sync.dma_start(out=xc0[:, 0:2, 0:512], in_=xTr0[:, 0:2, 0:512])
        nc.sync.dma_start(out=xc0[:, 2:6, 0:512], in_=xTr0[:, 2:6, 0:512])
        ballt = consts.tile([128, 4], f32, tag="ball", name="ball")
        nc.sync.dma_start(out=ballt[:], in_=ball[:])
        for k in range(1, KT):
            nc.sync.dma_start(out=wall_t[k][:],
                              in_=wall[k * 128:(k + 1) * 128, :])
        w1_t = [t[:, 0:128] for t in wall_t]
        w2_t = [t[:, 128:256] for t in wall_t]
        w3_t = [t[:, 256:384] for t in wall_t]
        bias = {f"b{i + 1}": ballt[:, i:i + 1] for i in range(4)}
        wpt = consts.tile([HD, 2 * DIM], bf16, tag="wpall", name="wpall")
        nc.sync.dma_start(out=wpt[:], in_=wpall[:])
        wp_t = {"A": wpt[:, 0:DIM], "B": wpt[:, DIM:2 * DIM]}

        # ---- persistent on-chip tensors ----
        # QAB rows 0:64 = C1-scaled Q^T(A); rows 64:128 = C1-scaled Q^T(B)
        QAB = persist.tile([128, S], bf16, tag="QAB", name="QAB")
        KAB = persist.tile([128, S], bf16, tag="KAB", name="KAB")
        # V_t[:, kt, 0:64]=V_A, col 64=ones, 65:129=V_B, col 129=ones
        V_t = persist.tile([128, NKT, 130], bf16, tag="V", name="V")
        nc.vector.tensor_copy(V_t[:, :, HD], ones32[:])
        nc.vector.tensor_copy(V_t[:, :, 65 + HD], ones32[:])

        # ---- QKV projections: x^T streamed once in [128,1024] chunks ----
        xTr = xT.rearrange("(k p) q -> p k q", p=128)

        def copy_dve(dst, src, b):
            nc.vector.tensor_scalar_add(dst, src, b)

        def copy_act(dst, src, b):
            nc.scalar.activation(dst, src, IDENT, bias=b, scale=1.0)

        def chunk_dma(qc):
            o = qc * 1024
            if qc == 0:
                xc = xc0  # first half already in flight
            else:
                xc = xcp.tile([128, KT, 1024], bf16, tag="xc", name="xc")
                nc.sync.dma_start(out=xc[:, :, 0:512],
                                  in_=xTr[:, :, o:o + 512])
            nc.sync.dma_start(out=xc[:, :, 512:1024],
                              in_=xTr[:, :, o + 512:o + 1024])
            return xc

        def main_chunk(qc, xc):  # qc in 0..3, covers q-cols 1024*qc..+1024
            for h in range(2):
                cs = slice(qc * 1024 + h * 512, qc * 1024 + (h + 1) * 512)
                hs = slice(h * 512, (h + 1) * 512)
                ps1 = ps_gp.tile([128, 512], f32, tag="gp", name="gp1")
                for k in range(KT):
                    nc.tensor.matmul(ps1[:], w1_t[k], xc[:, k, hs],
                                     start=(k == 0), stop=(k == KT - 1))
                copy_dve(QAB[0:HD, cs], ps1[0:HD, :], bias["b1"][0:HD, :])
                copy_act(KAB[HD:128, cs], ps1[HD:128, :],
                         bias["b1"][HD:128, :])
                ps2 = ps_gp.tile([128, 512], f32, tag="gp", name="gp2")
                for k in range(KT):
                    nc.tensor.matmul(ps2[:], w2_t[k], xc[:, k, hs],
                                     start=(k == 0), stop=(k == KT - 1))
                vt = vtw.tile([128, 512], bf16, tag="vt", name="vt")
                copy_act(KAB[0:HD, cs], ps2[0:HD, :], bias["b2"][0:HD, :])
                copy_dve(vt[HD:128, :], ps2[HD:128, :], bias["b2"][HD:128, :])
                ps3 = ps_gp.tile([128, 512], f32, tag="gp", name="gp3")
                for k in range(KT):
                    nc.tensor.matmul(ps3[:], w3_t[k], xc[:, k, hs],
                                     start=(k == 0), stop=(k == KT - 1))
                copy_dve(vt[0:HD, :], ps3[0:HD, :], bias["b3"][0:HD, :])
                copy_act(QAB[HD:128, cs], ps3[HD:128, :],
                         bias["b3"][HD:128, :])
                tp = ps_gp.tile([128, 512], bf16, tag="gp", name="gp4")
                for t4 in range(4):
                    ts_ = slice(t4 * 128, (t4 + 1) * 128)
                    nc.tensor.transpose(tp[:, ts_], vt[:, ts_], identb[:])
                kt0 = qc * 8 + h * 4
                tpr = tp[:].rearrange("p (t c) -> p t c", t=4)
                nc.scalar.copy(V_t[:, kt0:kt0 + 4, 0:HD], tpr[:, :, 0:HD])
                nc.vector.tensor_copy(V_t[:, kt0:kt0 + 4, 65:65 + HD],
                                      tpr[:, :, HD:128])

        # ---- attention super-sweeps + fused projection ----
        pid = nc.sync.partition_id()
        QBloc = persist.tile([128, SH], bf16, tag="QBloc", name="QBloc")
        vo = {"A": 0, "B": 65}
        rowsl = {"A": slice(0, HD), "B": slice(HD, 128)}
        tpos = {"A": (0, 0), "B": (64, 0)}
        sweeps = [("A", 0, "A", 1), ("A", 2, "B", 0), ("A", 3, "B", 1),
                  ("A", 4, "B", 2), ("A", 5, "B", 3), ("A", 6, "A", 7)]

        hold = {"pending": [], "pvq": []}

        def finish_units(jobs, out_ps):
            # normalize NOW (frees out_ps for the next sweep), defer the rest
            rc = normp.tile([128, 8], f32, tag="rc", name="rc")
            nc.vector.reciprocal(rc[:], out_ps[:, :, HD])
            # at is hd-padded to 128 so the XBAR transpose below sees a
            # 128-multiple free dim; cols 64:128 are never-read garbage
            at = atp.tile([128, 8, 128], bf16, tag="at", name="at")
            nc.vector.tensor_tensor(
                at[:, :, 0:HD], out_ps[:, :, 0:HD],
                rc[:].rearrange("p (a b) -> p a b", b=1).broadcast_to(
                    [128, 8, HD]),
                MUL)
            # (out_ps cols 65:128 are dead padding - keeps PV chunks off
            # PSUM bank boundaries)

            units = []
            cells = [{} for _ in jobs]

            def mk_tr(i, j):
                def u():
                    atT = ps_gp.tile([HD, 4, 128], bf16, tag="gp", name="atT")
                    for c in range(4):
                        nc.tensor.transpose(atT[:, c, :],
                                            at[:, i * 4 + c, 0:HD],
                                            identb[:])
                    cells[i]["atT"] = atT
                return u

            def mk_atm(i, j):
                def u():
                    atm = atmp.tile([HD, 512], bf16, tag="atm", name="atm")
                    nc.scalar.copy(
                        atm[:].rearrange("p (t c) -> p t c", t=4),
                        cells[i]["atT"][:])
                    cp = outp.tile([128, KT, 512], bf16, tag="cp", name="cp")
                    cells[i]["atm"] = atm
                    cells[i]["cp"] = cp
                return u

            def mk_proj(i, j, m):
                def u():
                    atm, cp = cells[i]["atm"], cells[i]["cp"]
                    pp = ps_gp.tile([128, 512], f32, tag="gp", name="pp")
                    nc.tensor.matmul(pp[:], wp_t[j][:, m * 128:(m + 1) * 128],
                                     atm[:], start=True, stop=True)
                    if m % 2 == 1 or (m == 2 and i == 0):
                        nc.scalar.copy(cp[:, m, :], pp[:])
                    else:
                        nc.vector.tensor_copy(cp[:, m, :], pp[:])
                return u

            def mk_dma(i, j, q, m0):
                def u():
                    ydram = yA if j == "A" else yB
                    ydr = ydram.rearrange("(m p) q -> p m q", p=128)
                    nc.sync.dma_start(
                        out=ydr[:, m0:m0 + 2, q * 512:(q + 1) * 512],
                        in_=cells[i]["cp"][:, m0:m0 + 2, :])
                return u

            for i, (j, q) in enumerate(jobs):
                units.append(mk_tr(i, j))
                units.append(mk_atm(i, j))
            for m in range(KT):
                for i, (j, q) in enumerate(jobs):
                    units.append(mk_proj(i, j, m))
                if m % 2 == 1 and m < KT - 1:
                    for i, (j, q) in enumerate(jobs):
                        units.append(mk_dma(i, j, q, m - 1))
            for i, (j, q) in enumerate(jobs):
                units.append(mk_dma(i, j, q, KT - 2))
            return units

        def begin_sweep(jobs):
            out_ps = ps_o.tile([128, 8, 128], f32, tag="out", name="out")
            return {"jobs": jobs, "out": out_ps, "kt": 0}

        def emit_kts(ss, n):
            jobs, out_ps = ss["jobs"], ss["out"]
            for _ in range(n):
                kt = ss["kt"]
                sts = []
                for i, (j, q) in enumerate(jobs):
                    st = ps_st.tile([128, 512], f32, tag="st",
                                    name=f"st{i}")
                    qsrc = QBloc if j == "B" else QAB
                    nc.tensor.matmul(st[:],
                                     KAB[rowsl[j], kt * 128:(kt + 1) * 128],
                                     qsrc[rowsl[j], q * 512:(q + 1) * 512],
                                     start=True, stop=True,
                                     tile_position=tpos[j])
                    sts.append(st)
                pt = ptp.tile([128, 1024], bf16, tag="pt", name="pt")
                # job0 -> scalar engine (true exp); job1 -> DVE bit trick
                nc.scalar.activation(pt[:, 0:512], sts[0][:],
                                     EXP, bias=bcol[:], scale=1.0 / C1)
                nc.vector.tensor_scalar_add(
                    pt[:, 512:1024].bitcast(i16), sts[1][:], C2V)

                def pv(kt=kt, pt=pt):
                    # out_ps slots share 2KB PSUM zero-regions (4 slots per
                    # bank): only the first slot of each bank may raise
                    # start_tensor_calc (it arms/zeroes the whole region) and
                    # only the last slot stops it.
                    for c in range(8):
                        j = jobs[c // 4][0]
                        nc.tensor.matmul(out_ps[:, c, 0:65],
                                         pt[:, c * 128:(c + 1) * 128],
                                         V_t[:, kt, vo[j]:vo[j] + 65],
                                         start=(kt == 0 and c % 4 == 0),
                                         stop=(kt == NKT - 1 and c % 4 == 3),
                                         skip_group_check=True)
                hold["pvq"].append(pv)
                while len(hold["pvq"]) > 11:
                    hold["pvq"].pop(0)()
                ss["kt"] += 1
                if hold["pending"]:
                    hold["pending"].pop(0)()

        def end_sweep(ss):
            # defer normalize + unit work behind the in-flight PVs so the
            # next sweep's QK/convert stream overlaps this sweep's tail
            def norm_then_units(ss=ss):
                hold["pending"].extend(
                    finish_units(ss["jobs"], ss["out"]))
            hold["pvq"].append(norm_then_units)

        # fused QKV + sweep 0 (x chunk DMAs prefetched one chunk ahead)
        ss0 = begin_sweep([(sweeps[0][0], sweeps[0][1]),
                           (sweeps[0][2], sweeps[0][3])])
        xcs = [chunk_dma(0), chunk_dma(1)]
        main_chunk(0, xcs[0])
        xcs.append(chunk_dma(2))
        emit_kts(ss0, 8)
        main_chunk(1, xcs[1])
        xcs.append(chunk_dma(3))
        emit_kts(ss0, 8)
        main_chunk(2, xcs[2])
        emit_kts(ss0, 8)
        main_chunk(3, xcs[3])
        if SIM_STATIC:
            nc.sync.dma_start(out=QBloc[HD:128, :], in_=QAB[HD:128, 0:SH])
        else:
            with tc.If((pid & 1) < 1) as cmp:
                nc.sync.dma_start(out=QBloc[HD:128, :], in_=QAB[HD:128, 0:SH])
            with cmp.Else():
                nc.sync.dma_start(out=QBloc[HD:128, :],
                                  in_=QAB[HD:128, SH:2 * SH])
        emit_kts(ss0, 8)
        end_sweep(ss0)

        for si, (j0, q0, j1, q1) in enumerate(sweeps[1:]):
            ss = begin_sweep([(j0, q0), (j1, q1)])
            emit_kts(ss, NKT)
            end_sweep(ss)
        while hold["pvq"]:
            hold["pvq"].pop(0)()
        while hold["pending"]:
            hold["pending"].pop(0)()


def _get_nc():
    if "nc" not in _CACHE:
        _CACHE["nc"] = _build_nc()
    return _CACHE["nc"]


def kernel(x, w_qkv, b_qkv, w_proj, b_proj):
    from concourse.bass_utils import run_bass_kernel_spmd

    BF = ml_dtypes.bfloat16
    x = np.asarray(x, dtype=np.float32)
    w_qkv = np.asarray(w_qkv, dtype=np.float32)
    b_qkv = np.asarray(b_qkv, dtype=np.float32)
    w_proj = np.asarray(w_proj, dtype=np.float32)
    b_proj = np.asarray(b_proj, dtype=np.float32)

    B = x.shape[0]
    xT = np.ascontiguousarray(x[0].T).astype(BF)  # [768, 4096]
    QS = SCALE * C1

    def wcol(block, h):
        o = block * DIM + h * HD
        return w_qkv[:, o:o + HD]

    def bcol_(block, h):
        o = block * DIM + h * HD
        return b_qkv[o:o + HD]

    in_maps = []
    meta = []
    z64 = np.zeros(HD, dtype=np.float32)
    for c in range(NCORES):
        hA, hB, qh = c, 8 + c // 2, c % 2
        m = {
            "xT": xT,
            "wall": np.concatenate(
                [wcol(0, hA) * QS, wcol(1, hB), wcol(1, hA), wcol(2, hB),
                 wcol(2, hA), wcol(0, hB) * QS], axis=1).astype(BF),
            "ball": np.stack(
                [np.concatenate([bcol_(0, hA) * QS, bcol_(1, hB)]),
                 np.concatenate([bcol_(1, hA), bcol_(2, hB)]),
                 np.concatenate([bcol_(2, hA), bcol_(0, hB) * QS]),
                 np.concatenate([z64, z64])], axis=1).astype(np.float32),
            "wpall": np.concatenate(
                [w_proj[hA * HD:(hA + 1) * HD, :],
                 w_proj[hB * HD:(hB + 1) * HD, :]], axis=1).astype(BF),
        }
        in_maps.append({k: np.ascontiguousarray(v) for k, v in m.items()})
        meta.append(qh)

    nc = _get_nc()
    res = run_bass_kernel_spmd(nc, in_maps, core_ids=list(range(NCORES)))

    Y = np.zeros((DIM, S), dtype=np.float64)
    for c in range(NCORES):
        Y += res.results[c]["yA"].astype(np.float64)
        qh = meta[c]
        Y[:, qh * SH:(qh + 1) * SH] += res.results[c]["yB"].astype(np.float64)
    out = (Y.T + b_proj.astype(np.float64)).astype(np.float32)
    return out.reshape(B, S, DIM)


# revision 64
# speedup vs baseline: 1.0015x; 1.0002x over previous
"""Multi-head self-attention (B=1, S=4096, DIM=768, H=12) on 8 Trainium2
NeuronCores.

Sharding: tensor-parallel over heads. Core c computes
  - full attention for head hA = c            (heads 0..7, all 4096 queries)
  - half attention for head hB = 8 + c//2     (heads 8..11, query half c%2)
Each core computes its own K/V projections for its two heads from x^T
streamed through SBUF once, runs attention fully on-chip, applies its heads'
slice of the output projection, and returns transposed partial projections
(bf16). The host sums per-core partials (the tensor-parallel all-reduce),
adds b_proj, and transposes back.

Fast path vs the fp32 baseline:
  - All matmuls in bf16 (x, QKV weights, Q/K/V, pt, proj weights).
  - exp(score) is split across two engines BY JOB: the scalar engine
    computes true exp for the sweep's first job; the vector engine handles
    the second job with a Schraudolph bit trick: Q is pre-scaled by
    C1 = 128/ln2 so the QK matmul emits scores in bf16-bits domain, then one
    tensor_scalar_add(+16261.5) with int16 output writes exp(s)*1.03 bit
    patterns directly into the bf16 pt tile (rel err ~3%, cancels in
    softmax's numerator/denominator except per-weight ripple). Each job's
    scores live in their own 1-bank PSUM tile (4-deep rotation) and PV
    consumption is deferred two k-tiles so neither engine's latency sits on
    the PE critical path.
  - PV is "flipped": stationary = pt q-chunk [128,128], moving = [V|1]
    [128,65], so each matmul streams only 65 output columns (the cost model
    charges output free size). Output lands q-on-partitions with the softmax
    denominator in column 64, so normalization is a per-partition reciprocal
    + broadcast multiply (no partition broadcast), then a cheap PE transpose
    restores [hd, q] for the output projection.
"""

import numpy as np
import ml_dtypes

DIM = 768
HEADS = 12
HD = 64
SCALE = HD ** (-0.5)
S = 4096
SH = 2048
NCORES = 8
KT = DIM // 128   # 6 k-tiles over the 768 contraction dim
NKT = S // 128    # 32 k-tiles over the 4096 sequence dim

C1 = 128.0 / np.log(2.0)   # schraudolph scale, folded into Q weights
C2V = 16256.0 + 5.5        # bf16 bits of 1.0 + centering, added at convert

_CACHE: dict = {}
SIM_STATIC = False  # profile scripts set True: TimelineSim can't eval branches


def _build_nc():
    import concourse.bacc as bacc
    import concourse.tile as tile
    from concourse import mybir
    from concourse.masks import make_identity

    f32 = mybir.dt.float32
    bf16 = mybir.dt.bfloat16
    i16 = mybir.dt.int16
    EXP = mybir.ActivationFunctionType.Exp
    IDENT = mybir.ActivationFunctionType.Identity

    nc = bacc.Bacc("TRN2", target_bir_lowering=False)

    # ---- DRAM I/O (per-core) ----
    xT = nc.dram_tensor("xT", [DIM, S], bf16, kind="ExternalInput")
    # packed: [:,0:128]=[wqA*C1*s|wkB], [:,128:256]=[wkA|wvB],
    #         [:,256:384]=[wvA|wqB*C1*s]
    wall = nc.dram_tensor("wall", [DIM, 384], bf16, kind="ExternalInput")
    # cols: b1=[bqA*C1*s;bkB], b2=[bkA;bvB], b3=[bvA;bqB*C1*s], b4=0
    ball = nc.dram_tensor("ball", [128, 4], f32, kind="ExternalInput")
    wpall = nc.dram_tensor("wpall", [HD, 2 * DIM], bf16, kind="ExternalInput")
    yA = nc.dram_tensor("yA", [DIM, S], bf16, kind="ExternalOutput")
    yB = nc.dram_tensor("yB", [DIM, SH], bf16, kind="ExternalOutput")

    with tile.TileContext(nc) as tc:
        _emit(nc, tc, mybir, make_identity, f32, bf16, i16, EXP, IDENT,
              xT, wall, ball, wpall, yA, yB)

    nc.compile()
    return nc


def _emit(nc, tc, mybir, make_identity, f32, bf16, i16, EXP, IDENT,
          xT, wall, ball, wpall, yA, yB):
    MUL = mybir.AluOpType.mult

    with tc.tile_pool(name="consts", bufs=1) as consts, \
         tc.tile_pool(name="persist", bufs=1) as persist, \
         tc.tile_pool(name="xcp", bufs=3) as xcp, \
         tc.tile_pool(name="vtw", bufs=2) as vtw, \
         tc.tile_pool(name="ptp", bufs=20) as ptp, \
         tc.tile_pool(name="atp", bufs=3) as atp, \
         tc.tile_pool(name="atmp", bufs=4) as atmp, \
         tc.tile_pool(name="normp", bufs=3) as normp, \
         tc.tile_pool(name="outp", bufs=5) as outp, \
         tc.tile_pool(name="ps_gp", bufs=2, space="PSUM") as ps_gp, \
         tc.tile_pool(name="ps_st", bufs=4, space="PSUM") as ps_st, \
         tc.tile_pool(name="ps_o", bufs=1, space="PSUM") as ps_o:

        # ---- constants & weights ----
        identb = consts.tile([128, 128], bf16, tag="ident", name="ident")
        make_identity(nc, identb[:])
        ones32 = consts.tile([128, NKT], bf16, tag="ones32", name="ones32")
        nc.vector.memset(ones32[:], 1.0)
        # match the bit-trick's 2^(5.5/128) scale so both exp paths agree
        bcol = consts.tile([128, 1], f32, tag="bcol", name="bcol")
        nc.vector.memset(bcol[:], float(np.log(2.0) * 5.5 / 128.0))

        # startup critical path: first matmul needs wall[0] + x k-tiles 0:2
        wall_t = [consts.tile([128, 384], bf16, tag=f"wall{k}",
                              name=f"wall{k}") for k in range(KT)]
        nc.sync.dma_start(out=wall_t[0][:], in_=wall[0:128, :])
        xc0 = xcp.tile([128, KT, 1024], bf16, tag="xc", name="xc")
        xTr0 = xT.rearrange("(k p) q -> p k q", p=128)
        nc.sync.dma_start(out=xc0[:, 0:2, 0:512], in_=xTr0[:, 0:2, 0:512])
        nc.sync.dma_start(out=xc0[:, 2:6, 0:512], in_=xTr0[:, 2:6, 0:512])
        ballt = consts.tile([128, 4], f32, tag="ball", name="ball")
        nc.sync.dma_start(out=ballt[:], in_=ball[:])
        for k in range(1, KT):
            nc.sync.dma_start(out=wall_t[k][:],
                              in_=wall[k * 128:(k + 1) * 128, :])
        w1_t = [t[:, 0:128] for t in wall_t]
        w2_t = [t[:, 128:256] for t in wall_t]
        w3_t = [t[:, 256:384] for t in wall_t]
        bias = {f"b{i + 1}": ballt[:, i:i + 1] for i in range(4)}
        wpt = consts.tile([HD, 2 * DIM], bf16, tag="wpall", name="wpall")
        nc.sync.dma_start(out=wpt[:], in_=wpall[:])
        wp_t = {"A": wpt[:, 0:DIM], "B": wpt[:, DIM:2 * DIM]}

        # ---- persistent on-chip tensors ----
        # QAB rows 0:64 = C1-scaled Q^T(A); rows 64:128 = C1-scaled Q^T(B)
        QAB = persist.tile([128, S], bf16, tag="QAB", name="QAB")
        KAB = persist.tile([128, S], bf16, tag="KAB", name="KAB")
        # V_t[:, kt, 0:64]=V_A, col 64=ones, 65:129=V_B, col 129=ones
        V_t = persist.tile([128, NKT, 130], bf16, tag="V", name="V")
        nc.vector.tensor_copy(V_t[:, :, HD], ones32[:])
        nc.vector.tensor_copy(V_t[:, :, 65 + HD], ones32[:])

        # ---- QKV projections: x^T streamed once in [128,1024] chunks ----
        xTr = xT.rearrange("(k p) q -> p k q", p=128)

        def copy_dve(dst, src, b):
            nc.vector.tensor_scalar_add(dst, src, b)

        def copy_act(dst, src, b):
            nc.scalar.activation(dst, src, IDENT, bias=b, scale=1.0)

        def chunk_dma(qc):
            o = qc * 1024
            if qc == 0:
                xc = xc0  # first half already in flight
                nc.sync.dma_start(out=xc[:, 0:2, 512:1024],
                                  in_=xTr[:, 0:2, o + 512:o + 1024])
                nc.sync.dma_start(out=xc[:, 2:6, 512:1024],
                                  in_=xTr[:, 2:6, o + 512:o + 1024])
            else:
                xc = xcp.tile([128, KT, 1024], bf16, tag="xc", name="xc")
                nc.sync.dma_start(out=xc[:, :, 0:512],
                                  in_=xTr[:, :, o:o + 512])
                nc.sync.dma_start(out=xc[:, :, 512:1024],
                                  in_=xTr[:, :, o + 512:o + 1024])
            return xc

        def main_chunk(qc, xc):  # qc in 0..3, covers q-cols 1024*qc..+1024
            for h in range(2):
                cs = slice(qc * 1024 + h * 512, qc * 1024 + (h + 1) * 512)
                hs = slice(h * 512, (h + 1) * 512)
                ps1 = ps_gp.tile([128, 512], f32, tag="gp", name="gp1")
                for k in range(KT):
                    nc.tensor.matmul(ps1[:], w1_t[k], xc[:, k, hs],
                                     start=(k == 0), stop=(k == KT - 1))
                copy_dve(QAB[0:HD, cs], ps1[0:HD, :], bias["b1"][0:HD, :])
                copy_act(KAB[HD:128, cs], ps1[HD:128, :],
                         bias["b1"][HD:128, :])
                ps2 = ps_gp.tile([128, 512], f32, tag="gp", name="gp2")
                for k in range(KT):
                    nc.tensor.matmul(ps2[:], w2_t[k], xc[:, k, hs],
                                     start=(k == 0), stop=(k == KT - 1))
                vt = vtw.tile([128, 512], bf16, tag="vt", name="vt")
                copy_act(KAB[0:HD, cs], ps2[0:HD, :], bias["b2"][0:HD, :])
                copy_dve(vt[HD:128, :], ps2[HD:128, :], bias["b2"][HD:128, :])
                ps3 = ps_gp.tile([128, 512], f32, tag="gp", name="gp3")
                for k in range(KT):
                    nc.tensor.matmul(ps3[:], w3_t[k], xc[:, k, hs],
                                     start=(k == 0), stop=(k == KT - 1))
                copy_dve(vt[0:HD, :], ps3[0:HD, :], bias["b3"][0:HD, :])
                copy_act(QAB[HD:128, cs], ps3[HD:128, :],
                         bias["b3"][HD:128, :])
                tp = ps_gp.tile([128, 512], bf16, tag="gp", name="gp4")
                for t4 in range(4):
                    ts_ = slice(t4 * 128, (t4 + 1) * 128)
                    nc.tensor.transpose(tp[:, ts_], vt[:, ts_], identb[:])
                kt0 = qc * 8 + h * 4
                tpr = tp[:].rearrange("p (t c) -> p t c", t=4)
                nc.scalar.copy(V_t[:, kt0:kt0 + 4, 0:HD], tpr[:, :, 0:HD])
                nc.vector.tensor_copy(V_t[:, kt0:kt0 + 4, 65:65 + HD],
                                      tpr[:, :, HD:128])

        # ---- attention super-sweeps + fused projection ----
        pid = nc.sync.partition_id()
        QBloc = persist.tile([128, SH], bf16, tag="QBloc", name="QBloc")
        vo = {"A": 0, "B": 65}
        rowsl = {"A": slice(0, HD), "B": slice(HD, 128)}
        tpos = {"A": (0, 0), "B": (64, 0)}
        sweeps = [("A", 0, "A", 1), ("A", 2, "B", 0), ("A", 3, "B", 1),
                  ("A", 4, "B", 2), ("A", 5, "B", 3), ("A", 6, "A", 7)]

        hold = {"pending": [], "pvq": []}

        def finish_units(jobs, out_ps):
            # normalize NOW (frees out_ps for the next sweep), defer the rest
            rc = normp.tile([128, 8], f32, tag="rc", name="rc")
            nc.vector.reciprocal(rc[:], out_ps[:, :, HD])
            # at is hd-padded to 128 so the XBAR transpose below sees a
            # 128-multiple free dim; cols 64:128 are never-read garbage
            at = atp.tile([128, 8, 128], bf16, tag="at", name="at")
            nc.vector.tensor_tensor(
                at[:, :, 0:HD], out_ps[:, :, 0:HD],
                rc[:].rearrange("p (a b) -> p a b", b=1).broadcast_to(
                    [128, 8, HD]),
                MUL)
            # (out_ps cols 65:128 are dead padding - keeps PV chunks off
            # PSUM bank boundaries)

            units = []
            cells = [{} for _ in jobs]

            def mk_tr(i, j):
                def u():
                    atT = ps_gp.tile([HD, 4, 128], bf16, tag="gp", name="atT")
                    for c in range(4):
                        nc.tensor.transpose(atT[:, c, :],
                                            at[:, i * 4 + c, 0:HD],
                                            identb[:])
                    cells[i]["atT"] = atT
                return u

            def mk_atm(i, j):
                def u():
                    atm = atmp.tile([HD, 512], bf16, tag="atm", name="atm")
                    nc.scalar.copy(
                        atm[:].rearrange("p (t c) -> p t c", t=4),
                        cells[i]["atT"][:])
                    cp = outp.tile([128, KT, 512], bf16, tag="cp", name="cp")
                    cells[i]["atm"] = atm
                    cells[i]["cp"] = cp
                return u

            def mk_proj(i, j, m):
                def u():
                    atm, cp = cells[i]["atm"], cells[i]["cp"]
                    pp = ps_gp.tile([128, 512], f32, tag="gp", name="pp")
                    nc.tensor.matmul(pp[:], wp_t[j][:, m * 128:(m + 1) * 128],
                                     atm[:], start=True, stop=True)
                    if m % 2 == 1 or (m == 2 and i == 0):
                        nc.scalar.copy(cp[:, m, :], pp[:])
                    else:
                        nc.vector.tensor_copy(cp[:, m, :], pp[:])
                return u

            def mk_dma(i, j, q, m0):
                def u():
                    ydram = yA if j == "A" else yB
                    ydr = ydram.rearrange("(m p) q -> p m q", p=128)
                    nc.sync.dma_start(
                        out=ydr[:, m0:m0 + 2, q * 512:(q + 1) * 512],
                        in_=cells[i]["cp"][:, m0:m0 + 2, :])
                return u

            for i, (j, q) in enumerate(jobs):
                units.append(mk_tr(i, j))
                units.append(mk_atm(i, j))
            for m in range(KT):
                for i, (j, q) in enumerate(jobs):
                    units.append(mk_proj(i, j, m))
                if m % 2 == 1 and m < KT - 1:
                    for i, (j, q) in enumerate(jobs):
                        units.append(mk_dma(i, j, q, m - 1))
            for i, (j, q) in enumerate(jobs):
                units.append(mk_dma(i, j, q, KT - 2))
            return units

        def begin_sweep(jobs):
            out_ps = ps_o.tile([128, 8, 128], f32, tag="out", name="out")
            return {"jobs": jobs, "out": out_ps, "kt": 0}

        def emit_kts(ss, n):
            jobs, out_ps = ss["jobs"], ss["out"]
            for _ in range(n):
                kt = ss["kt"]
                sts = []
                for i, (j, q) in enumerate(jobs):
                    st = ps_st.tile([128, 512], f32, tag="st",
                                    name=f"st{i}")
                    qsrc = QBloc if j == "B" else QAB
                    nc.tensor.matmul(st[:],
                                     KAB[rowsl[j], kt * 128:(kt + 1) * 128],
                                     qsrc[rowsl[j], q * 512:(q + 1) * 512],
                                     start=True, stop=True,
                                     tile_position=tpos[j])
                    sts.append(st)
                pt = ptp.tile([128, 1024], bf16, tag="pt", name="pt")
                # job0 -> scalar engine (true exp); job1 -> DVE bit trick
                nc.scalar.activation(pt[:, 0:512], sts[0][:],
                                     EXP, bias=bcol[:], scale=1.0 / C1)
                nc.vector.tensor_scalar_add(
                    pt[:, 512:1024].bitcast(i16), sts[1][:], C2V)

                def pv(kt=kt, pt=pt):
                    # out_ps slots share 2KB PSUM zero-regions (4 slots per
                    # bank): only the first slot of each bank may raise
                    # start_tensor_calc (it arms/zeroes the whole region) and
                    # only the last slot stops it.
                    for c in range(8):
                        j = jobs[c // 4][0]
                        nc.tensor.matmul(out_ps[:, c, 0:65],
                                         pt[:, c * 128:(c + 1) * 128],
                                         V_t[:, kt, vo[j]:vo[j] + 65],
                                         start=(kt == 0 and c % 4 == 0),
                                         stop=(kt == NKT - 1 and c % 4 == 3),
                                         skip_group_check=True)
                hold["pvq"].append(pv)
                while len(hold["pvq"]) > 11:
                    hold["pvq"].pop(0)()
                ss["kt"] += 1
                if hold["pending"]:
                    hold["pending"].pop(0)()

        def end_sweep(ss):
            # defer normalize + unit work behind the in-flight PVs so the
            # next sweep's QK/convert stream overlaps this sweep's tail
            def norm_then_units(ss=ss):
                hold["pending"].extend(
                    finish_units(ss["jobs"], ss["out"]))
            hold["pvq"].append(norm_then_units)

        # fused QKV + sweep 0 (x chunk DMAs prefetched one chunk ahead)
        ss0 = begin_sweep([(sweeps[0][0], sweeps[0][1]),
                           (sweeps[0][2], sweeps[0][3])])
        xcs = [chunk_dma(0), chunk_dma(1)]
        main_chunk(0, xcs[0])
        xcs.append(chunk_dma(2))
        emit_kts(ss0, 8)
        main_chunk(1, xcs[1])
        xcs.append(chunk_dma(3))
        emit_kts(ss0, 8)
        main_chunk(2, xcs[2])
        emit_kts(ss0, 8)
        main_chunk(3, xcs[3])
        if SIM_STATIC:
            nc.sync.dma_start(out=QBloc[HD:128, :], in_=QAB[HD:128, 0:SH])
        else:
            with tc.If((pid & 1) < 1) as cmp:
                nc.sync.dma_start(out=QBloc[HD:128, :], in_=QAB[HD:128, 0:SH])
            with cmp.Else():
                nc.sync.dma_start(out=QBloc[HD:128, :],
                                  in_=QAB[HD:128, SH:2 * SH])
        emit_kts(ss0, 8)
        end_sweep(ss0)

        for si, (j0, q0, j1, q1) in enumerate(sweeps[1:]):
            ss = begin_sweep([(j0, q0), (j1, q1)])
            emit_kts(ss, NKT)
            end_sweep(ss)
        while hold["pvq"]:
            hold["pvq"].pop(0)()
        while hold["pending"]:
            hold["pending"].pop(0)()


def _get_nc():
    if "nc" not in _CACHE:
        _CACHE["nc"] = _build_nc()
    return _CACHE["nc"]


def kernel(x, w_qkv, b_qkv, w_proj, b_proj):
    from concourse.bass_utils import run_bass_kernel_spmd

    BF = ml_dtypes.bfloat16
    x = np.asarray(x, dtype=np.float32)
    w_qkv = np.asarray(w_qkv, dtype=np.float32)
    b_qkv = np.asarray(b_qkv, dtype=np.float32)
    w_proj = np.asarray(w_proj, dtype=np.float32)
    b_proj = np.asarray(b_proj, dtype=np.float32)

    B = x.shape[0]
    xT = np.ascontiguousarray(x[0].T).astype(BF)  # [768, 4096]
    QS = SCALE * C1

    def wcol(block, h):
        o = block * DIM + h * HD
        return w_qkv[:, o:o + HD]

    def bcol_(block, h):
        o = block * DIM + h * HD
        return b_qkv[o:o + HD]

    in_maps = []
    meta = []
    z64 = np.zeros(HD, dtype=np.float32)
    for c in range(NCORES):
        hA, hB, qh = c, 8 + c // 2, c % 2
        m = {
            "xT": xT,
            "wall": np.concatenate(
                [wcol(0, hA) * QS, wcol(1, hB), wcol(1, hA), wcol(2, hB),
                 wcol(2, hA), wcol(0, hB) * QS], axis=1).astype(BF),
            "ball": np.stack(
                [np.concatenate([bcol_(0, hA) * QS, bcol_(1, hB)]),
                 np.concatenate([bcol_(1, hA), bcol_(2, hB)]),
                 np.concatenate([bcol_(2, hA), bcol_(0, hB) * QS]),
                 np.concatenate([z64, z64])], axis=1).astype(np.float32),
            "wpall": np.concatenate(
                [w_proj[hA * HD:(hA + 1) * HD, :],
                 w_proj[hB * HD:(hB + 1) * HD, :]], axis=1).astype(BF),
        }
        in_maps.append({k: np.ascontiguousarray(v) for k, v in m.items()})
        meta.append(qh)

    nc = _get_nc()
    res = run_bass_kernel_spmd(nc, in_maps, core_ids=list(range(NCORES)))

    Y = np.zeros((DIM, S), dtype=np.float64)
    for c in range(NCORES):
        Y += res.results[c]["yA"].astype(np.float64)
        qh = meta[c]
        Y[:, qh * SH:(qh + 1) * SH] += res.results[c]["yB"].astype(np.float64)
    out = (Y.T + b_proj.astype(np.float64)).astype(np.float32)
    return out.reshape(B, S, DIM)


# revision 72
# speedup vs baseline: 1.0041x; 1.0026x over previous
"""Multi-head self-attention (B=1, S=4096, DIM=768, H=12) on 8 Trainium2
NeuronCores.

Sharding: tensor-parallel over heads. Core c computes
  - full attention for head hA = c            (heads 0..7, all 4096 queries)
  - half attention for head hB = 8 + c//2     (heads 8..11, query half c%2)
Each core computes its own K/V projections for its two heads from x^T
streamed through SBUF once, runs attention fully on-chip, applies its heads'
slice of the output projection, and returns transposed partial projections
(bf16). The host sums per-core partials (the tensor-parallel all-reduce),
adds b_proj, and transposes back.

Fast path vs the fp32 baseline:
  - All matmuls in bf16 (x, QKV weights, Q/K/V, pt, proj weights).
  - exp(score) is split across two engines BY JOB: the scalar engine
    computes true exp for the sweep's first job; the vector engine handles
    the second job with a Schraudolph bit trick: Q is pre-scaled by
    C1 = 128/ln2 so the QK matmul emits scores in bf16-bits domain, then one
    tensor_scalar_add(+16261.5) with int16 output writes exp(s)*1.03 bit
    patterns directly into the bf16 pt tile (rel err ~3%, cancels in
    softmax's numerator/denominator except per-weight ripple). Each job's
    scores live in their own 1-bank PSUM tile (4-deep rotation) and PV
    consumption is deferred two k-tiles so neither engine's latency sits on
    the PE critical path.
  - PV is "flipped": stationary = pt q-chunk [128,128], moving = [V|1]
    [128,65], so each matmul streams only 65 output columns (the cost model
    charges output free size). Output lands q-on-partitions with the softmax
    denominator in column 64, so normalization is a per-partition reciprocal
    + broadcast multiply (no partition broadcast), then a cheap PE transpose
    restores [hd, q] for the output projection.
"""

import numpy as np
import ml_dtypes

DIM = 768
HEADS = 12
HD = 64
SCALE = HD ** (-0.5)
S = 4096
SH = 2048
NCORES = 8
KT = DIM // 128   # 6 k-tiles over the 768 contraction dim
NKT = S // 128    # 32 k-tiles over the 4096 sequence dim

C1 = 128.0 / np.log(2.0)   # schraudolph scale, folded into Q weights
C2V = 16256.0 + 5.5        # bf16 bits of 1.0 + centering, added at convert

_CACHE: dict = {}
SIM_STATIC = False  # profile scripts set True: TimelineSim can't eval branches


def _build_nc():
    import concourse.bacc as bacc
    import concourse.tile as tile
    from concourse import mybir
    from concourse.masks import make_identity

    f32 = mybir.dt.float32
    bf16 = mybir.dt.bfloat16
    i16 = mybir.dt.int16
    EXP = mybir.ActivationFunctionType.Exp
    IDENT = mybir.ActivationFunctionType.Identity

    nc = bacc.Bacc("TRN2", target_bir_lowering=False)

    # ---- DRAM I/O (per-core) ----
    xT = nc.dram_tensor("xT", [DIM, S], bf16, kind="ExternalInput")
    # packed: [:,0:128]=[wqA*C1*s|wkB], [:,128:256]=[wkA|wvB],
    #         [:,256:384]=[wvA|wqB*C1*s]
    wall = nc.dram_tensor("wall", [DIM, 384], bf16, kind="ExternalInput")
    # cols: b1=[bqA*C1*s;bkB], b2=[bkA;bvB], b3=[bvA;bqB*C1*s], b4=0
    ball = nc.dram_tensor("ball", [128, 4], f32, kind="ExternalInput")
    wpall = nc.dram_tensor("wpall", [HD, 2 * DIM], bf16, kind="ExternalInput")
    yA = nc.dram_tensor("yA", [DIM, S], bf16, kind="ExternalOutput")
    yB = nc.dram_tensor("yB", [DIM, SH], bf16, kind="ExternalOutput")

    with tile.TileContext(nc) as tc:
        _emit(nc, tc, mybir, make_identity, f32, bf16, i16, EXP, IDENT,
              xT, wall, ball, wpall, yA, yB)

    nc.compile()
    return nc


def _emit(nc, tc, mybir, make_identity, f32, bf16, i16, EXP, IDENT,
          xT, wall, ball, wpall, yA, yB):
    MUL = mybir.AluOpType.mult

    with tc.tile_pool(name="consts", bufs=1) as consts, \
         tc.tile_pool(name="persist", bufs=1) as persist, \
         tc.tile_pool(name="xcp", bufs=3) as xcp, \
         tc.tile_pool(name="vtw", bufs=2) as vtw, \
         tc.tile_pool(name="ptp", bufs=20) as ptp, \
         tc.tile_pool(name="atp", bufs=3) as atp, \
         tc.tile_pool(name="atmp", bufs=4) as atmp, \
         tc.tile_pool(name="normp", bufs=3) as normp, \
         tc.tile_pool(name="outp", bufs=5) as outp, \
         tc.tile_pool(name="ps_gp", bufs=2, space="PSUM") as ps_gp, \
         tc.tile_pool(name="ps_st", bufs=4, space="PSUM") as ps_st, \
         tc.tile_pool(name="ps_o", bufs=1, space="PSUM") as ps_o:

        # ---- constants & weights ----
        identb = consts.tile([128, 128], bf16, tag="ident", name="ident")
        make_identity(nc, identb[:])
        ones32 = consts.tile([128, NKT], bf16, tag="ones32", name="ones32")
        nc.vector.memset(ones32[:], 1.0)
        # match the bit-trick's 2^(5.5/128) scale so both exp paths agree
        bcol = consts.tile([128, 1], f32, tag="bcol", name="bcol")
        nc.vector.memset(bcol[:], float(np.log(2.0) * 5.5 / 128.0))

        # startup critical path: first matmul needs wall[0] + x k-tiles 0:2
        wall_t = [consts.tile([128, 384], bf16, tag=f"wall{k}",
                              name=f"wall{k}") for k in range(KT)]
        nc.sync.dma_start(out=wall_t[0][:], in_=wall[0:128, :])
        xc0 = xcp.tile([128, KT, 1024], bf16, tag="xc", name="xc")
        xTr0 = xT.rearrange("(k p) q -> p k q", p=128)
        nc.sync.dma_start(out=xc0[:, 0:2, 0:512], in_=xTr0[:, 0:2, 0:512])
        nc.sync.dma_start(out=xc0[:, 2:6, 0:512], in_=xTr0[:, 2:6, 0:512])
        ballt = consts.tile([128, 4], f32, tag="ball", name="ball")
        nc.sync.dma_start(out=ballt[:], in_=ball[:])
        for k in range(1, KT):
            nc.sync.dma_start(out=wall_t[k][:],
                              in_=wall[k * 128:(k + 1) * 128, :])
        w1_t = [t[:, 0:128] for t in wall_t]
        w2_t = [t[:, 128:256] for t in wall_t]
        w3_t = [t[:, 256:384] for t in wall_t]
        bias = {f"b{i + 1}": ballt[:, i:i + 1] for i in range(4)}
        wpt = consts.tile([HD, 2 * DIM], bf16, tag="wpall", name="wpall")
        nc.sync.dma_start(out=wpt[:], in_=wpall[:])
        wp_t = {"A": wpt[:, 0:DIM], "B": wpt[:, DIM:2 * DIM]}

        # ---- persistent on-chip tensors ----
        # QAB rows 0:64 = C1-scaled Q^T(A); rows 64:128 = C1-scaled Q^T(B)
        QAB = persist.tile([128, S], bf16, tag="QAB", name="QAB")
        KAB = persist.tile([128, S], bf16, tag="KAB", name="KAB")
        # V_t[:, kt, 0:64]=V_A, col 64=ones, 65:129=V_B, col 129=ones
        V_t = persist.tile([128, NKT, 130], bf16, tag="V", name="V")
        nc.vector.tensor_copy(V_t[:, :, HD], ones32[:])
        nc.vector.tensor_copy(V_t[:, :, 65 + HD], ones32[:])

        # ---- QKV projections: x^T streamed once in [128,1024] chunks ----
        xTr = xT.rearrange("(k p) q -> p k q", p=128)

        def copy_dve(dst, src, b):
            nc.vector.tensor_scalar_add(dst, src, b)

        def copy_act(dst, src, b):
            nc.scalar.activation(dst, src, IDENT, bias=b, scale=1.0)

        def chunk_dma(qc):
            o = qc * 1024
            if qc == 0:
                xc = xc0  # first half already in flight
                nc.sync.dma_start(out=xc[:, 0:2, 512:1024],
                                  in_=xTr[:, 0:2, o + 512:o + 1024])
                nc.sync.dma_start(out=xc[:, 2:6, 512:1024],
                                  in_=xTr[:, 2:6, o + 512:o + 1024])
            else:
                xc = xcp.tile([128, KT, 1024], bf16, tag="xc", name="xc")
                nc.sync.dma_start(out=xc[:, :, 0:512],
                                  in_=xTr[:, :, o:o + 512])
                nc.sync.dma_start(out=xc[:, :, 512:1024],
                                  in_=xTr[:, :, o + 512:o + 1024])
            return xc

        def main_chunk(qc, xc):  # qc in 0..3, covers q-cols 1024*qc..+1024
            for h in range(2):
                cs = slice(qc * 1024 + h * 512, qc * 1024 + (h + 1) * 512)
                hs = slice(h * 512, (h + 1) * 512)
                ps1 = ps_gp.tile([128, 512], f32, tag="gp", name="gp1")
                for k in range(KT):
                    nc.tensor.matmul(ps1[:], w1_t[k], xc[:, k, hs],
                                     start=(k == 0), stop=(k == KT - 1))
                copy_act(QAB[0:HD, cs], ps1[0:HD, :], bias["b1"][0:HD, :])
                copy_act(KAB[HD:128, cs], ps1[HD:128, :],
                         bias["b1"][HD:128, :])
                ps2 = ps_gp.tile([128, 512], f32, tag="gp", name="gp2")
                for k in range(KT):
                    nc.tensor.matmul(ps2[:], w2_t[k], xc[:, k, hs],
                                     start=(k == 0), stop=(k == KT - 1))
                vt = vtw.tile([128, 512], bf16, tag="vt", name="vt")
                copy_act(KAB[0:HD, cs], ps2[0:HD, :], bias["b2"][0:HD, :])
                copy_dve(vt[HD:128, :], ps2[HD:128, :], bias["b2"][HD:128, :])
                ps3 = ps_gp.tile([128, 512], f32, tag="gp", name="gp3")
                for k in range(KT):
                    nc.tensor.matmul(ps3[:], w3_t[k], xc[:, k, hs],
                                     start=(k == 0), stop=(k == KT - 1))
                copy_dve(vt[0:HD, :], ps3[0:HD, :], bias["b3"][0:HD, :])
                copy_act(QAB[HD:128, cs], ps3[HD:128, :],
                         bias["b3"][HD:128, :])
                tp = ps_gp.tile([128, 512], bf16, tag="gp", name="gp4")
                for t4 in range(4):
                    ts_ = slice(t4 * 128, (t4 + 1) * 128)
                    nc.tensor.transpose(tp[:, ts_], vt[:, ts_], identb[:])
                kt0 = qc * 8 + h * 4
                tpr = tp[:].rearrange("p (t c) -> p t c", t=4)
                nc.scalar.copy(V_t[:, kt0:kt0 + 4, 0:HD], tpr[:, :, 0:HD])
                nc.vector.tensor_copy(V_t[:, kt0:kt0 + 4, 65:65 + HD],
                                      tpr[:, :, HD:128])

        # ---- attention super-sweeps + fused projection ----
        pid = nc.sync.partition_id()
        QBloc = persist.tile([128, SH], bf16, tag="QBloc", name="QBloc")
        vo = {"A": 0, "B": 65}
        rowsl = {"A": slice(0, HD), "B": slice(HD, 128)}
        tpos = {"A": (0, 0), "B": (64, 0)}
        sweeps = [("A", 0, "A", 1), ("A", 2, "B", 0), ("A", 3, "B", 1),
                  ("A", 4, "B", 2), ("A", 5, "B", 3), ("A", 6, "A", 7)]

        hold = {"pending": [], "pvq": []}

        def finish_units(jobs, out_ps):
            # normalize NOW (frees out_ps for the next sweep), defer the rest
            rc = normp.tile([128, 8], f32, tag="rc", name="rc")
            nc.vector.reciprocal(rc[:], out_ps[:, :, HD])
            # at is hd-padded to 128 so the XBAR transpose below sees a
            # 128-multiple free dim; cols 64:128 are never-read garbage
            at = atp.tile([128, 8, 128], bf16, tag="at", name="at")
            nc.vector.tensor_tensor(
                at[:, :, 0:HD], out_ps[:, :, 0:HD],
                rc[:].rearrange("p (a b) -> p a b", b=1).broadcast_to(
                    [128, 8, HD]),
                MUL)
            # (out_ps cols 65:128 are dead padding - keeps PV chunks off
            # PSUM bank boundaries)

            units = []
            cells = [{} for _ in jobs]

            def mk_tr(i, j):
                def u():
                    atT = ps_gp.tile([HD, 4, 128], bf16, tag="gp", name="atT")
                    for c in range(4):
                        nc.tensor.transpose(atT[:, c, :],
                                            at[:, i * 4 + c, 0:HD],
                                            identb[:])
                    cells[i]["atT"] = atT
                return u

            def mk_atm(i, j):
                def u():
                    atm = atmp.tile([HD, 512], bf16, tag="atm", name="atm")
                    nc.scalar.copy(
                        atm[:].rearrange("p (t c) -> p t c", t=4),
                        cells[i]["atT"][:])
                    cp = outp.tile([128, KT, 512], bf16, tag="cp", name="cp")
                    cells[i]["atm"] = atm
                    cells[i]["cp"] = cp
                return u

            def mk_proj(i, j, m):
                def u():
                    atm, cp = cells[i]["atm"], cells[i]["cp"]
                    pp = ps_gp.tile([128, 512], f32, tag="gp", name="pp")
                    nc.tensor.matmul(pp[:], wp_t[j][:, m * 128:(m + 1) * 128],
                                     atm[:], start=True, stop=True)
                    if m % 2 == 1 or (m == 2 and i == 0):
                        nc.scalar.copy(cp[:, m, :], pp[:])
                    else:
                        nc.vector.tensor_copy(cp[:, m, :], pp[:])
                return u

            def mk_dma(i, j, q, m0):
                def u():
                    ydram = yA if j == "A" else yB
                    ydr = ydram.rearrange("(m p) q -> p m q", p=128)
                    nc.sync.dma_start(
                        out=ydr[:, m0:m0 + 2, q * 512:(q + 1) * 512],
                        in_=cells[i]["cp"][:, m0:m0 + 2, :])
                return u

            for i, (j, q) in enumerate(jobs):
                units.append(mk_tr(i, j))
                units.append(mk_atm(i, j))
            for m in range(KT):
                for i, (j, q) in enumerate(jobs):
                    units.append(mk_proj(i, j, m))
                if m % 2 == 1 and m < KT - 1:
                    for i, (j, q) in enumerate(jobs):
                        units.append(mk_dma(i, j, q, m - 1))
            for i, (j, q) in enumerate(jobs):
                units.append(mk_dma(i, j, q, KT - 2))
            return units

        def begin_sweep(jobs):
            out_ps = ps_o.tile([128, 8, 128], f32, tag="out", name="out")
            return {"jobs": jobs, "out": out_ps, "kt": 0}

        def emit_kts(ss, n):
            jobs, out_ps = ss["jobs"], ss["out"]
            for _ in range(n):
                kt = ss["kt"]
                sts = []
                for i, (j, q) in enumerate(jobs):
                    st = ps_st.tile([128, 512], f32, tag="st",
                                    name=f"st{i}")
                    qsrc = QBloc if j == "B" else QAB
                    nc.tensor.matmul(st[:],
                                     KAB[rowsl[j], kt * 128:(kt + 1) * 128],
                                     qsrc[rowsl[j], q * 512:(q + 1) * 512],
                                     start=True, stop=True,
                                     tile_position=tpos[j])
                    sts.append(st)
                pt = ptp.tile([128, 1024], bf16, tag="pt", name="pt")
                # job0 -> scalar engine (true exp); job1 -> DVE bit trick
                nc.scalar.activation(pt[:, 0:512], sts[0][:],
                                     EXP, bias=bcol[:], scale=1.0 / C1)
                nc.vector.tensor_scalar_add(
                    pt[:, 512:1024].bitcast(i16), sts[1][:], C2V)

                def pv(kt=kt, pt=pt):
                    # out_ps slots share 2KB PSUM zero-regions (4 slots per
                    # bank): only the first slot of each bank may raise
                    # start_tensor_calc (it arms/zeroes the whole region) and
                    # only the last slot stops it.
                    for c in range(8):
                        j = jobs[c // 4][0]
                        nc.tensor.matmul(out_ps[:, c, 0:65],
                                         pt[:, c * 128:(c + 1) * 128],
                                         V_t[:, kt, vo[j]:vo[j] + 65],
                                         start=(kt == 0 and c % 4 == 0),
                                         stop=(kt == NKT - 1 and c % 4 == 3),
                                         skip_group_check=True)
                hold["pvq"].append(pv)
                while len(hold["pvq"]) > 11:
                    hold["pvq"].pop(0)()
                ss["kt"] += 1
                if hold["pending"]:
                    hold["pending"].pop(0)()

        def end_sweep(ss):
            # defer normalize + unit work behind the in-flight PVs so the
            # next sweep's QK/convert stream overlaps this sweep's tail
            def norm_then_units(ss=ss):
                hold["pending"].extend(
                    finish_units(ss["jobs"], ss["out"]))
            hold["pvq"].append(norm_then_units)

        # fused QKV + sweep 0 (x chunk DMAs prefetched one chunk ahead)
        ss0 = begin_sweep([(sweeps[0][0], sweeps[0][1]),
                           (sweeps[0][2], sweeps[0][3])])
        xcs = [chunk_dma(0), chunk_dma(1)]
        main_chunk(0, xcs[0])
        xcs.append(chunk_dma(2))
        emit_kts(ss0, 8)
        main_chunk(1, xcs[1])
        xcs.append(chunk_dma(3))
        emit_kts(ss0, 8)
        main_chunk(2, xcs[2])
        emit_kts(ss0, 8)
        main_chunk(3, xcs[3])
        if SIM_STATIC:
            nc.sync.dma_start(out=QBloc[HD:128, :], in_=QAB[HD:128, 0:SH])
        else:
            with tc.If((pid & 1) < 1) as cmp:
                nc.sync.dma_start(out=QBloc[HD:128, :], in_=QAB[HD:128, 0:SH])
            with cmp.Else():
                nc.sync.dma_start(out=QBloc[HD:128, :],
                                  in_=QAB[HD:128, SH:2 * SH])
        emit_kts(ss0, 8)
        end_sweep(ss0)

        for si, (j0, q0, j1, q1) in enumerate(sweeps[1:]):
            ss = begin_sweep([(j0, q0), (j1, q1)])
            emit_kts(ss, NKT)
            end_sweep(ss)
        while hold["pvq"]:
            hold["pvq"].pop(0)()
        while hold["pending"]:
            hold["pending"].pop(0)()


def _get_nc():
    if "nc" not in _CACHE:
        _CACHE["nc"] = _build_nc()
    return _CACHE["nc"]


def kernel(x, w_qkv, b_qkv, w_proj, b_proj):
    from concourse.bass_utils import run_bass_kernel_spmd

    BF = ml_dtypes.bfloat16
    x = np.asarray(x, dtype=np.float32)
    w_qkv = np.asarray(w_qkv, dtype=np.float32)
    b_qkv = np.asarray(b_qkv, dtype=np.float32)
    w_proj = np.asarray(w_proj, dtype=np.float32)
    b_proj = np.asarray(b_proj, dtype=np.float32)

    B = x.shape[0]
    xT = np.ascontiguousarray(x[0].T).astype(BF)  # [768, 4096]
    QS = SCALE * C1

    def wcol(block, h):
        o = block * DIM + h * HD
        return w_qkv[:, o:o + HD]

    def bcol_(block, h):
        o = block * DIM + h * HD
        return b_qkv[o:o + HD]

    in_maps = []
    meta = []
    z64 = np.zeros(HD, dtype=np.float32)
    for c in range(NCORES):
        hA, hB, qh = c, 8 + c // 2, c % 2
        m = {
            "xT": xT,
            "wall": np.concatenate(
                [wcol(0, hA) * QS, wcol(1, hB), wcol(1, hA), wcol(2, hB),
                 wcol(2, hA), wcol(0, hB) * QS], axis=1).astype(BF),
            "ball": np.stack(
                [np.concatenate([bcol_(0, hA) * QS, bcol_(1, hB)]),
                 np.concatenate([bcol_(1, hA), bcol_(2, hB)]),
                 np.concatenate([bcol_(2, hA), bcol_(0, hB) * QS]),
                 np.concatenate([z64, z64])], axis=1).astype(np.float32),
            "wpall": np.concatenate(
                [w_proj[hA * HD:(hA + 1) * HD, :],
                 w_proj[hB * HD:(hB + 1) * HD, :]], axis=1).astype(BF),
        }
        in_maps.append({k: np.ascontiguousarray(v) for k, v in m.items()})
        meta.append(qh)

    nc = _get_nc()
    res = run_bass_kernel_spmd(nc, in_maps, core_ids=list(range(NCORES)))

    Y = np.zeros((DIM, S), dtype=np.float64)
    for c in range(NCORES):
        Y += res.results[c]["yA"].astype(np.float64)
        qh = meta[c]
        Y[:, qh * SH:(qh + 1) * SH] += res.results[c]["yB"].astype(np.float64)
    out = (Y.T + b_proj.astype(np.float64)).astype(np.float32)
    return out.reshape(B, S, DIM)
